# revision 33
# baseline (speedup 1.0000x reference)
"""RWKV-v4 block (time-mix WKV attention + channel-mix GLU) on 8 Trainium2
NeuronCores, data-parallel over batch B.  v3: all matmuls fp8e4m3 DoubleRow
(2 contraction chunks per instruction, ~2x bf16 column rate), bf16 WKV chain
in STT form on DVE, software-pipelined across the 4 local batches.

Layouts per core (B_local=4, T=1024, C=512, H=2048):
  - layout A: [t(128p), n(8), c(512)] -- LayerNorm (bn_stats), residual adds,
    final store.
  - layout B: [c(128p), cc(4), t(1056)] -- WKV scan along free dim, matmul
    operands.  A->B via bf16 DMA transpose through a DRAM bounce (32 zero
    cols in front make the token shift an offset view), then fp8 quantize.

Weight scaling (fp8 weights packed x64 except Wv x8):
  k_raw = 64k   -> e  = exp(k_raw/64 - ln64) = e_true/64        (ACT bias)
  v_raw = 8v    -> ev = e*v_raw = e_true*v/8
  P' = P/8 (scan ev), Q' = Q/64 (scan e)
  N' = eu*ev + P'_{t-1} = N/8,  D' = eu*e + Q'_{t-1} = D/64
  y' = N'/D' = 8y  -> srw = y'*sig in fp8 (|8y*sig| < 40 ok)
  att_raw = (64*Wo)@srw = 512*att -> x1 = xa + att_raw/512      (STT)
  r_raw = 64r -> sigmoid(r_raw/64 + rb); cWk/cWr/cWv x64 likewise.
The time-mix token shift is folded into k/v/r weights (Wa = W*diag(g*tm),
Wb = W*diag(g*(1-tm))), the shifted moving operand is an offset view of the
same fp8 tile.  Channel-mix shifts stay explicit (output dim >> contraction).
"""

import numpy as np
import ml_dtypes
from contextlib import ExitStack

import concourse.bass as bass
import concourse.tile as tile
from concourse import bacc, mybir

B, T, C = 32, 1024, 512
H = 4 * C
NCORES = 8
BL = B // NCORES  # batches per core
NT = T // 128     # 8 t-subtiles per batch
CC = C // 128     # 4 channel chunks
HC = H // 128     # 16 hidden chunks
SC = 64.0         # fp8 weight scale
ISC = 1.0 / SC
SCV = 8.0         # Wv fp8 scale
LN64 = float(np.log(64.0))

F32 = mybir.dt.float32
BF16 = mybir.dt.bfloat16
FP8 = mybir.dt.float8e4
AX = mybir.AxisListType
OP = mybir.AluOpType
AF = mybir.ActivationFunctionType
DR = mybir.MatmulPerfMode.DoubleRow


def _emit(nc, tc, ctx, io, bl):
    x_d = io["x"].ap()
    y_d = io["y"].ap()

    def col(name, c0):  # [128,1] slice of a [N] dram vector
        return io[name].ap()[c0 * 128:(c0 + 1) * 128].rearrange(
            "(c one) -> c one", one=1)

    sb = ctx.enter_context(tc.tile_pool(name="sb", bufs=1))
    ps2 = ctx.enter_context(tc.tile_pool(name="ps2", bufs=3, space="PSUM"))
    ps = ctx.enter_context(tc.tile_pool(name="ps", bufs=2, space="PSUM"))
    dramp = ctx.enter_context(tc.tile_pool(name="dram", bufs=4, space="DRAM"))

    # ---- small consts first (cheap), then x(b0) so LN1 starts immediately;
    # weights stream in behind it in first-use order ----
    def vecload(name, n=CC, madd=None):
        ts_ = []
        for i in range(n):
            t_ = sb.tile([128, 1], F32, tag=f"v_{name}_{i}")
            nc.gpsimd.dma_start(t_[:], col(name, i))
            if madd is not None:
                nc.vector.tensor_scalar_add(t_[:], t_[:], madd)
            ts_.append(t_)
        return ts_

    eps_t = sb.tile([128, 1], F32, tag="eps")
    nc.vector.memset(eps_t[:], 1e-5)
    nln64_t = sb.tile([128, 1], F32, tag="nln64")
    nc.vector.memset(nln64_t[:], -LN64)
    ident = sb.tile([128, 128], F32, tag="ident")
    nc.gpsimd.dma_start(ident[:], io["ident512"].ap())
    zrow = sb.tile([32, C], BF16, tag="zrow")
    nc.vector.memset(zrow[:], 0.0)

    delta_c = vecload("delta")
    eu_c = vecload("eu")
    rb_c = vecload("rb")
    kkb_c = vecload("kkb", HC)
    cmk_c = vecload("cmk", madd=-1.0)
    cmr_c = vecload("cmr", madd=-1.0)

    # ---- per-batch pools ----
    xa_pool = ctx.enter_context(tc.tile_pool(name="xa", bufs=2))
    lnp = ctx.enter_context(tc.tile_pool(name="ln", bufs=1))
    bq = ctx.enter_context(tc.tile_pool(name="bq", bufs=1))
    bp = ctx.enter_context(tc.tile_pool(name="bp", bufs=2))
    b2p = ctx.enter_context(tc.tile_pool(name="b2p", bufs=1))
    wkvp = ctx.enter_context(tc.tile_pool(name="wkv", bufs=2))
    srwp = ctx.enter_context(tc.tile_pool(name="srw", bufs=1))
    cmp_ = ctx.enter_context(tc.tile_pool(name="cm", bufs=1))
    outp = ctx.enter_context(tc.tile_pool(name="out", bufs=2))

    xa_t = {}     # b -> [128, NT, 512] f32 (becomes x1 in place after Wo)
    xnB8_t = {}   # b -> [128, CC, 1056] fp8 (LN1, k/v/r folded matmuls)
    xk2_t = {}    # b -> [128, CC, T] fp8
    xr2_t = {}    # b -> [128, CC, T] fp8
    srw_t = {}    # b -> [128, CC, T] fp8

    def layer_norm(b, src_tile, which):
        """[128, NT, 512] f32 layout A -> xnB bf16 [128, CC, 1056] layout B
        (32 zero cols in front for the token shift)."""
        bnst = lnp.tile([128, NT, 6], F32, tag=f"bnst{which}")
        for n in range(NT):
            nc.vector.bn_stats(bnst[:, n, :], src_tile[:, n, :])
        mv = lnp.tile([128, NT, 2], F32, tag=f"mv{which}")
        for n in range(NT):
            nc.vector.bn_aggr(mv[:, n, :], bnst[:, n, :])
        sqv = lnp.tile([128, NT], F32, tag=f"sqv{which}")
        nc.scalar.activation(sqv[:], mv[:, :, 1], AF.Sqrt, bias=eps_t[:])
        rstd = lnp.tile([128, NT], F32, tag=f"rstd{which}")
        nc.vector.reciprocal(rstd[:], sqv[:])
        xn = lnp.tile([128, NT, 512], BF16, tag=f"xn{which}")
        for n in range(NT):
            nc.vector.tensor_scalar(xn[:, n, :], src_tile[:, n, :],
                                    mv[:, n, 0:1], rstd[:, n:n + 1],
                                    op0=OP.subtract, op1=OP.mult)
        xnd = dramp.tile([T + 32, C], BF16, tag=f"xnd{which}")
        nc.sync.dma_start(xnd[0:32, :], zrow[:])
        nc.sync.dma_start(xnd[32:T + 32].rearrange("(n p) c -> p n c", p=128),
                          xn[:])
        pool = bq if which == 1 else b2p
        xnB = pool.tile([128, CC, T + 32], BF16, tag=f"xnB{which}")
        for cc in range(CC):
            nc.sync.dma_start_transpose(xnB[:, cc, :],
                                        xnd[:, cc * 128:(cc + 1) * 128])
        return xnB

    def stage_A(b):
        """load x(b), LN1(b), fp8 quantize."""
        xb = x_d[b].rearrange("(n p) c -> p n c", p=128)
        xa = xa_pool.tile([128, NT, 512], F32, tag="xa", name=f"xa{b}")
        nc.sync.dma_start(xa[:], xb)
        xa_t[b] = xa
        xnB = layer_norm(b, xa, 1)
        xnB8 = bp.tile([128, CC, T + 32], FP8, tag="xnB8", name=f"xnB8_{b}")
        for cc in range(CC):
            nc.gpsimd.tensor_scalar_mul(xnB8[:, cc, :], xnB[:, cc, :], 1.0)
        xnB8_t[b] = xnB8

    # ---- weights (resident), loaded after batch-0's x ----
    def load_w(name, d1, d2, dt):
        t_ = sb.tile([128, d1, d2], dt, tag=f"w_{name}")
        nc.gpsimd.dma_start(t_[:], io[name].ap())
        return t_

    def stage_weights_tm():
        global wka, wkb, wva, wvb, wra, wrb, woT8
        wka = load_w("wka", CC, C, FP8)
        wkb = load_w("wkb", CC, C, FP8)
        wva = load_w("wva", CC, C, FP8)
        wvb = load_w("wvb", CC, C, FP8)
        wra = load_w("wra", CC, C, FP8)
        wrb = load_w("wrb", CC, C, FP8)
        woT8 = load_w("woT8", CC, C, FP8)

    def stage_weights_cm():
        global cwk8, cwv8, cwr8
        cwk8 = load_w("cwk8", CC, H, FP8)
        cwv8 = load_w("cwv8", HC, C, FP8)
        cwr8 = load_w("cwr8", CC, C, FP8)

    def dr_fold(out_ps, wa, wb, xnB8, hh, th):
        """k/v/r: 4 DoubleRow matmuls, contraction 1024 = (aligned 512 +
        shifted 512), accumulating into out_ps [128, 512]."""
        t0 = 32 + th * 512
        for j in range(2):
            nc.tensor.matmul(out_ps[:], wa[:, 2 * j:2 * j + 2,
                                           hh * 128:(hh + 1) * 128],
                             xnB8[:, 2 * j:2 * j + 2, t0:t0 + 512],
                             start=(j == 0), stop=False, perf_mode=DR)
        for j in range(2):
            nc.tensor.matmul(out_ps[:], wb[:, 2 * j:2 * j + 2,
                                           hh * 128:(hh + 1) * 128],
                             xnB8[:, 2 * j:2 * j + 2, t0 - 1:t0 + 511],
                             start=False, stop=(j == 1), perf_mode=DR)

    def stage_K(b):
        """k/v/r matmuls + WKV chains -> srw(b) [128, CC, T] fp8."""
        xnB8 = xnB8_t[b]
        srw = srwp.tile([128, CC, T], FP8, tag="srw", name=f"srw{b}")
        for hh in range(CC):
            k_ps = ps2.tile([128, 1024], F32, tag="ps2", name=f"kps{b}_{hh}")
            for th in range(2):
                dr_fold(k_ps[:, th * 512:(th + 1) * 512], wka, wkb,
                        xnB8, hh, th)
            e_t = wkvp.tile([128, T], BF16, tag="e")
            nc.scalar.activation(e_t[:], k_ps[:], AF.Exp, scale=ISC,
                                 bias=nln64_t[:])
            r_ps = ps2.tile([128, 1024], F32, tag="ps2", name=f"rps{b}_{hh}")
            for th in range(2):
                dr_fold(r_ps[:, th * 512:(th + 1) * 512], wra, wrb,
                        xnB8, hh, th)
            sig = wkvp.tile([128, T], BF16, tag="sig")
            nc.scalar.activation(sig[:], r_ps[:], AF.Sigmoid, scale=ISC,
                                 bias=rb_c[hh][:])
            v_ps = ps2.tile([128, 1024], F32, tag="ps2", name=f"vps{b}_{hh}")
            for th in range(2):
                dr_fold(v_ps[:, th * 512:(th + 1) * 512], wva, wvb,
                        xnB8, hh, th)
            vcp = wkvp.tile([128, T], BF16, tag="vcp")
            nc.scalar.activation(vcp[:], v_ps[:], AF.Copy)
            ev = wkvp.tile([128, T], BF16, tag="ev")
            nc.gpsimd.tensor_tensor(ev[:], e_t[:], vcp[:], op=OP.mult)
            Pb = wkvp.tile([128, T + 1], BF16, tag="Pb")
            Qb = wkvp.tile([128, T + 1], F32, tag="Qb")
            nc.vector.memset(Pb[:, 0:1], 0.0)
            nc.vector.memset(Qb[:, 0:1], 0.0)
            db = delta_c[hh][:].to_broadcast((128, T))
            nc.vector.tensor_tensor_scan(Pb[:, 1:T + 1], db, ev[:],
                                         0.0, op0=OP.mult, op1=OP.add)
            nc.vector.tensor_tensor_scan(Qb[:, 1:T + 1], db, e_t[:],
                                         0.0, op0=OP.mult, op1=OP.add)
            # N' into ev, D' into Qb (slot t holds Q'_{t-1}), both in place
            nc.vector.scalar_tensor_tensor(ev[:], ev[:], eu_c[hh][:],
                                           Pb[:, 0:T], op0=OP.mult, op1=OP.add)
            nc.vector.scalar_tensor_tensor(Qb[:, 0:T], e_t[:], eu_c[hh][:],
                                           Qb[:, 0:T], op0=OP.mult, op1=OP.add)
            df = wkvp.tile([128, T], F32, tag="df")
            nc.vector.reciprocal_approx_fast(df[:], Qb[:, 0:T])
            nc.vector.scalar_tensor_tensor(ev[:], ev[:], 1.0, df[:],
                                           op0=OP.bypass, op1=OP.mult)
            nc.gpsimd.tensor_tensor(srw[:, hh, :], ev[:], sig[:], op=OP.mult)
        srw_t[b] = srw

    def stage_W(b):
        """Wo (fp8 DR, srw-stationary) + residual add in place: xa -> x1."""
        xa = xa_t[b]
        srw = srw_t[b]
        for n in range(NT):
            p_ = ps.tile([128, 512], F32, tag="ps", name=f"wops{b}_{n}")
            for j in range(2):
                nc.tensor.matmul(p_[:],
                                 srw[:, 2 * j:2 * j + 2,
                                     n * 128:(n + 1) * 128],
                                 woT8[:, 2 * j:2 * j + 2, :],
                                 start=(j == 0), stop=False, perf_mode=DR)
            # += 512*xa via identity matmul, then x1 = p/512 on ACT
            nc.tensor.matmul(p_[:], ident[:], xa[:, n, :],
                             start=False, stop=True)
            nc.scalar.activation(xa[:, n, :], p_[:], AF.Copy,
                                 scale=1.0 / 512.0)

    xn2B_t = {}

    def stage_L(b):
        """LN2 + xk2 mix (fp8); xr2 deferred to stage_L2."""
        xn2B = layer_norm(b, xa_t[b], 2)
        xn2B_t[b] = xn2B
        d2 = b2p.tile([128, CC, T], BF16, tag="d2", name=f"d2_{b}")
        xk2 = b2p.tile([128, CC, T], FP8, tag="xk2", name=f"xk2_{b}")
        for cc in range(CC):
            nc.gpsimd.tensor_tensor(d2[:, cc, :], xn2B[:, cc, 32:T + 32],
                                    xn2B[:, cc, 31:T + 31], op=OP.subtract)
        for cc in range(CC):
            nc.vector.scalar_tensor_tensor(xk2[:, cc, :], d2[:, cc, :],
                                           cmk_c[cc][:], xn2B[:, cc, 32:T + 32],
                                           op0=OP.mult, op1=OP.add)
        xk2_t[b] = xk2
        return d2

    def stage_L2(b, d2):
        xn2B = xn2B_t[b]
        xr2 = b2p.tile([128, CC, T], FP8, tag="xr2", name=f"xr2_{b}")
        for cc in range(CC):
            nc.vector.scalar_tensor_tensor(xr2[:, cc, :], d2[:, cc, :],
                                           cmr_c[cc][:], xn2B[:, cc, 32:T + 32],
                                           op0=OP.mult, op1=OP.add)
        xr2_t[b] = xr2

    def stage_M(b):
        """Channel mix: kk = relu(cWk xk2)^2 (fp8), rkv, residual, store."""
        xk2, xr2 = xk2_t[b], xr2_t[b]
        x1 = xa_t[b]
        yb = y_d[b].rearrange("(n p) c -> p n c", p=128)
        kk2 = cmp_.tile([128, HC, T], FP8, tag="kk2", name=f"kk2_{b}")
        rl = cmp_.tile([128, 2, T], BF16, tag="rl")
        for g in range(HC // 2):
            pps = []
            for u in range(2):
                hh = 2 * g + u
                p_ = ps2.tile([128, 1024], F32, tag="ps2",
                              name=f"kkps{b}_{hh}")
                for th in range(2):
                    for j in range(2):
                        nc.tensor.matmul(
                            p_[:, th * 512:(th + 1) * 512],
                            cwk8[:, 2 * j:2 * j + 2, hh * 128:(hh + 1) * 128],
                            xk2[:, 2 * j:2 * j + 2, th * 512:(th + 1) * 512],
                            start=(j == 0), stop=(j == 1), perf_mode=DR)
                pps.append((hh, p_))
            for u, (hh, p_) in enumerate(pps):
                nc.scalar.activation(rl[:, u, :], p_[:], AF.Relu, scale=ISC,
                                     bias=kkb_c[hh][:])
            for u, (hh, p_) in enumerate(pps):
                nc.gpsimd.tensor_tensor(kk2[:, hh, :], rl[:, u, :],
                                        rl[:, u, :], op=OP.mult)
        for n in range(NT):
            rp = ps.tile([128, 512], F32, tag="ps", name=f"rp{b}_{n}")
            for j in range(2):
                nc.tensor.matmul(rp[:],
                                 xr2[:, 2 * j:2 * j + 2,
                                     n * 128:(n + 1) * 128],
                                 cwr8[:, 2 * j:2 * j + 2, :],
                                 start=(j == 0), stop=(j == 1), perf_mode=DR)
            sig2 = outp.tile([128, 512], BF16, tag="sig2")
            nc.scalar.activation(sig2[:], rp[:], AF.Sigmoid, scale=ISC)
            kvp = ps.tile([128, 512], F32, tag="ps", name=f"kvp{b}_{n}")
            for j in range(HC // 2):
                nc.tensor.matmul(
                    kvp[:], kk2[:, 2 * j:2 * j + 2, n * 128:(n + 1) * 128],
                    cwv8[:, 2 * j:2 * j + 2, :],
                    start=(j == 0), stop=(j == HC // 2 - 1), perf_mode=DR)
            kvs = outp.tile([128, 512], F32, tag="kvs")
            nc.vector.tensor_tensor(kvs[:], kvp[:], sig2[:], op=OP.mult)
            nc.vector.scalar_tensor_tensor(kvs[:], kvs[:], ISC, x1[:, n, :],
                                           op0=OP.mult, op1=OP.add)
            nc.gpsimd.dma_start(yb[:, n, :], kvs[:])

    # ---- software pipeline over batches ----
    # PE queue order: K(0) | W(0) K(1) M(0) | W(1) K(2) M(1) | ... | M(bl-1)
    stage_A(0)
    stage_weights_tm()
    stage_weights_cm()
    stage_K(0)
    if bl > 1:
        stage_A(1)
    for b in range(bl):
        stage_W(b)
        d2 = stage_L(b)
        if b + 1 < bl:
            stage_K(b + 1)
        stage_L2(b, d2)
        if b + 2 < bl:
            stage_A(b + 2)
        stage_M(b)


def build_program(bl=BL):
    nc = bacc.Bacc("TRN2", target_bir_lowering=False, debug=False,
                   num_devices=NCORES)
    io = {}
    io["x"] = nc.dram_tensor("x", [bl, T, C], F32, kind="ExternalInput")
    io["y"] = nc.dram_tensor("y", [bl, T, C], F32, kind="ExternalOutput")
    for nm, d1, d2 in [("wka", CC, C), ("wkb", CC, C), ("wva", CC, C),
                       ("wvb", CC, C), ("wra", CC, C), ("wrb", CC, C),
                       ("woT8", CC, C), ("cwk8", CC, H), ("cwv8", HC, C),
                       ("cwr8", CC, C)]:
        io[nm] = nc.dram_tensor(nm, [128, d1, d2], FP8, kind="ExternalInput")
    for nm, n in [("delta", C), ("eu", C), ("rb", C),
                  ("cmk", C), ("cmr", C), ("kkb", H)]:
        io[nm] = nc.dram_tensor(nm, [n], F32, kind="ExternalInput")
    io["ident512"] = nc.dram_tensor("ident512", [128, 128], F32,
                                    kind="ExternalInput")

    with tile.TileContext(nc) as tc:
        with ExitStack() as ctx:
            _emit(nc, tc, ctx, io, bl)
    nc.compile()
    return nc


def _pack8(w, scale=SC):  # [C_in, M] f32 -> [128, C_in//128, M] fp8e4m3
    ci, m = w.shape
    w8 = np.clip(w * scale, -240.0, 240.0).astype(ml_dtypes.float8_e4m3)
    return np.ascontiguousarray(w8.reshape(ci // 128, 128, m).transpose(1, 0, 2))


def host_params(inputs):
    """Host-side parameter prep (O(C^2) only)."""
    f32 = np.float32
    g1 = np.asarray(inputs["ln1_g"], f32)
    b1 = np.asarray(inputs["ln1_b"], f32)
    g2 = np.asarray(inputs["ln2_g"], f32)
    b2 = np.asarray(inputs["ln2_b"], f32)
    Wk = np.asarray(inputs["Wk"], f32)
    Wv = np.asarray(inputs["Wv"], f32)
    Wr = np.asarray(inputs["Wr"], f32)
    Wo = np.asarray(inputs["Wo"], f32)
    cWk = np.asarray(inputs["cWk"], f32)
    cWr = np.asarray(inputs["cWr"], f32)
    cWv = np.asarray(inputs["cWv"], f32)
    tmk = np.asarray(inputs["tm_k"], f32)
    tmv = np.asarray(inputs["tm_v"], f32)
    tmr = np.asarray(inputs["tm_r"], f32)

    # biases from LN betas land inside ACT bias terms; only the zero case is
    # supported (true for this model's init)
    assert np.allclose(Wk @ b1, 0.0, atol=1e-30), "nonzero ln1_b unsupported"
    assert np.allclose(Wv @ b1, 0.0, atol=1e-30), "nonzero ln1_b unsupported"
    assert np.allclose(cWr @ b2, 0.0, atol=1e-30), "nonzero ln2_b unsupported"

    p = {
        "wka": _pack8(Wk.T * (g1 * tmk)[:, None]),
        "wkb": _pack8(Wk.T * (g1 * (1.0 - tmk))[:, None]),
        "wva": _pack8(Wv.T * (g1 * tmv)[:, None], SCV),
        "wvb": _pack8(Wv.T * (g1 * (1.0 - tmv))[:, None], SCV),
        "wra": _pack8(Wr.T * (g1 * tmr)[:, None]),
        "wrb": _pack8(Wr.T * (g1 * (1.0 - tmr))[:, None]),
        "woT8": _pack8(Wo.T),
        "cwk8": _pack8(cWk.T * g2[:, None]),
        "cwv8": _pack8(cWv.T),
        "cwr8": _pack8(cWr.T * g2[:, None]),
        "delta": np.exp(-np.exp(np.asarray(inputs["time_decay"], f32))),
        "eu": np.exp(np.asarray(inputs["time_first"], f32)),
        "cmk": np.asarray(inputs["cm_k"], f32),
        "cmr": np.asarray(inputs["cm_r"], f32),
        "rb": (Wr @ b1).astype(f32),
        "kkb": (cWk @ b2).astype(f32),
        "ident512": (np.eye(128, dtype=f32) * 512.0),
    }
    return p


_CACHE = {}


def kernel(**inputs):
    from concourse.bass_utils import run_bass_kernel_spmd

    if "nc" not in _CACHE:
        _CACHE["nc"] = build_program(BL)
    nc = _CACHE["nc"]

    p = host_params(inputs)
    x = np.asarray(inputs["x"], np.float32)
    in_maps = []
    for c in range(NCORES):
        m = dict(p)
        m["x"] = np.ascontiguousarray(x[c * BL:(c + 1) * BL])
        in_maps.append(m)
    res = run_bass_kernel_spmd(nc, in_maps, list(range(NCORES)))
    out = np.concatenate([res.results[c]["y"] for c in range(NCORES)], axis=0)
    return out.astype(np.float32)


# revision 35
# speedup vs baseline: 1.6041x; 1.6041x over previous
"""RWKV-v4 block (time-mix WKV attention + channel-mix GLU) on 8 Trainium2
NeuronCores, data-parallel over batch B.  v3: all matmuls fp8e4m3 DoubleRow
(2 contraction chunks per instruction, ~2x bf16 column rate), bf16 WKV chain
in STT form on DVE, software-pipelined across the 4 local batches.

Layouts per core (B_local=4, T=1024, C=512, H=2048):
  - layout A: [t(128p), n(8), c(512)] -- LayerNorm (bn_stats), residual adds,
    final store.
  - layout B: [c(128p), cc(4), t(1056)] -- WKV scan along free dim, matmul
    operands.  A->B via bf16 DMA transpose through a DRAM bounce (32 zero
    cols in front make the token shift an offset view), then fp8 quantize.

Weight scaling (fp8 weights packed x64 except Wv x8):
  k_raw = 64k   -> e  = exp(k_raw/64 - ln64) = e_true/64        (ACT bias)
  v_raw = 8v    -> ev = e*v_raw = e_true*v/8
  P' = P/8 (scan ev), Q' = Q/64 (scan e)
  N' = eu*ev + P'_{t-1} = N/8,  D' = eu*e + Q'_{t-1} = D/64
  y' = N'/D' = 8y  -> srw = y'*sig in fp8 (|8y*sig| < 40 ok)
  att_raw = (64*Wo)@srw = 512*att -> x1 = xa + att_raw/512      (STT)
  r_raw = 64r -> sigmoid(r_raw/64 + rb); cWk/cWr/cWv x64 likewise.
The time-mix token shift is folded into k/v/r weights (Wa = W*diag(g*tm),
Wb = W*diag(g*(1-tm))), the shifted moving operand is an offset view of the
same fp8 tile.  Channel-mix shifts stay explicit (output dim >> contraction).
"""

import numpy as np
import ml_dtypes
from contextlib import ExitStack

import concourse.bass as bass
import concourse.tile as tile
from concourse import bacc, mybir

B, T, C = 32, 1024, 512
H = 4 * C
NCORES = 8
BL = B // NCORES  # batches per core
NT = T // 128     # 8 t-subtiles per batch
CC = C // 128     # 4 channel chunks
HC = H // 128     # 16 hidden chunks
SC = 64.0         # fp8 weight scale
ISC = 1.0 / SC
SCV = 8.0         # Wv fp8 scale
LN64 = float(np.log(64.0))

F32 = mybir.dt.float32
BF16 = mybir.dt.bfloat16
FP8 = mybir.dt.float8e4
AX = mybir.AxisListType
OP = mybir.AluOpType
AF = mybir.ActivationFunctionType
DR = mybir.MatmulPerfMode.DoubleRow


def _emit(nc, tc, ctx, io, bl):
    x_d = io["x"].ap()
    y_d = io["y"].ap()

    def col(name, c0):  # [128,1] slice of a [N] dram vector
        return io[name].ap()[c0 * 128:(c0 + 1) * 128].rearrange(
            "(c one) -> c one", one=1)

    sb = ctx.enter_context(tc.tile_pool(name="sb", bufs=1))
    ps2 = ctx.enter_context(tc.tile_pool(name="ps2", bufs=3, space="PSUM"))
    ps = ctx.enter_context(tc.tile_pool(name="ps", bufs=2, space="PSUM"))
    dramp = ctx.enter_context(tc.tile_pool(name="dram", bufs=4, space="DRAM"))

    # ---- small consts first (cheap), then x(b0) so LN1 starts immediately;
    # weights stream in behind it in first-use order ----
    def vecload(name, n=CC, madd=None):
        ts_ = []
        for i in range(n):
            t_ = sb.tile([128, 1], F32, tag=f"v_{name}_{i}")
            nc.gpsimd.dma_start(t_[:], col(name, i))
            if madd is not None:
                nc.vector.tensor_scalar_add(t_[:], t_[:], madd)
            ts_.append(t_)
        return ts_

    eps_t = sb.tile([128, 1], F32, tag="eps")
    nc.vector.memset(eps_t[:], 1e-5)
    nln64_t = sb.tile([128, 1], F32, tag="nln64")
    nc.vector.memset(nln64_t[:], -LN64)
    ident = sb.tile([128, 128], F32, tag="ident")
    nc.gpsimd.dma_start(ident[:], io["ident512"].ap())
    zrow = sb.tile([32, C], BF16, tag="zrow")
    nc.vector.memset(zrow[:], 0.0)

    delta_c = vecload("delta")
    eu_c = vecload("eu")
    rb_c = vecload("rb")
    kkb_c = vecload("kkb", HC)
    cmk_c = vecload("cmk", madd=-1.0)
    cmr_c = vecload("cmr", madd=-1.0)

    # ---- per-batch pools ----
    xa_pool = ctx.enter_context(tc.tile_pool(name="xa", bufs=2))
    lnp = ctx.enter_context(tc.tile_pool(name="ln", bufs=1))
    bq = ctx.enter_context(tc.tile_pool(name="bq", bufs=1))
    bp = ctx.enter_context(tc.tile_pool(name="bp", bufs=2))
    b2p = ctx.enter_context(tc.tile_pool(name="b2p", bufs=1))
    wkvp = ctx.enter_context(tc.tile_pool(name="wkv", bufs=2))
    srwp = ctx.enter_context(tc.tile_pool(name="srw", bufs=1))
    cmp_ = ctx.enter_context(tc.tile_pool(name="cm", bufs=1))
    outp = ctx.enter_context(tc.tile_pool(name="out", bufs=2))

    xa_t = {}     # b -> [128, NT, 512] f32 (becomes x1 in place after Wo)
    xnB8_t = {}   # b -> [128, CC, 1056] fp8 (LN1, k/v/r folded matmuls)
    xk2_t = {}    # b -> [128, CC, T] fp8
    xr2_t = {}    # b -> [128, CC, T] fp8
    srw_t = {}    # b -> [128, CC, T] fp8

    def layer_norm(b, src_tile, which):
        """[128, NT, 512] f32 layout A -> xnB bf16 [128, CC, 1056] layout B
        (32 zero cols in front for the token shift)."""
        bnst = lnp.tile([128, NT, 6], F32, tag=f"bnst{which}")
        for n in range(NT):
            nc.vector.bn_stats(bnst[:, n, :], src_tile[:, n, :])
        mv = lnp.tile([128, NT, 2], F32, tag=f"mv{which}")
        for n in range(NT):
            nc.vector.bn_aggr(mv[:, n, :], bnst[:, n, :])
        sqv = lnp.tile([128, NT], F32, tag=f"sqv{which}")
        nc.scalar.activation(sqv[:], mv[:, :, 1], AF.Sqrt, bias=eps_t[:])
        rstd = lnp.tile([128, NT], F32, tag=f"rstd{which}")
        nc.vector.reciprocal(rstd[:], sqv[:])
        xn = lnp.tile([128, NT, 512], BF16, tag=f"xn{which}")
        for n in range(NT):
            nc.vector.tensor_scalar(xn[:, n, :], src_tile[:, n, :],
                                    mv[:, n, 0:1], rstd[:, n:n + 1],
                                    op0=OP.subtract, op1=OP.mult)
        xnd = dramp.tile([T + 32, C], BF16, tag=f"xnd{which}")
        nc.sync.dma_start(xnd[0:32, :], zrow[:])
        nc.sync.dma_start(xnd[32:T + 32].rearrange("(n p) c -> p n c", p=128),
                          xn[:])
        pool = bq if which == 1 else b2p
        xnB = pool.tile([128, CC, T + 32], BF16, tag=f"xnB{which}")
        for cc in range(CC):
            nc.sync.dma_start_transpose(xnB[:, cc, :],
                                        xnd[:, cc * 128:(cc + 1) * 128])
        return xnB

    def stage_A(b):
        """load x(b), LN1(b), fp8 quantize."""
        xb = x_d[b].rearrange("(n p) c -> p n c", p=128)
        xa = xa_pool.tile([128, NT, 512], F32, tag="xa", name=f"xa{b}")
        nc.sync.dma_start(xa[:], xb)
        xa_t[b] = xa
        xnB = layer_norm(b, xa, 1)
        xnB8 = bp.tile([128, CC, T + 32], FP8, tag="xnB8", name=f"xnB8_{b}")
        for cc in range(CC):
            nc.scalar.activation(xnB8[:, cc, :], xnB[:, cc, :], AF.Copy)
        xnB8_t[b] = xnB8

    # ---- weights (resident), loaded after batch-0's x ----
    def load_w(name, d1, d2, dt):
        t_ = sb.tile([128, d1, d2], dt, tag=f"w_{name}")
        nc.gpsimd.dma_start(t_[:], io[name].ap())
        return t_

    def stage_weights_tm():
        global wka, wkb, wva, wvb, wra, wrb, woT8
        wka = load_w("wka", CC, C, FP8)
        wkb = load_w("wkb", CC, C, FP8)
        wva = load_w("wva", CC, C, FP8)
        wvb = load_w("wvb", CC, C, FP8)
        wra = load_w("wra", CC, C, FP8)
        wrb = load_w("wrb", CC, C, FP8)
        woT8 = load_w("woT8", CC, C, FP8)

    def stage_weights_cm():
        global cwk8, cwv8, cwr8
        cwk8 = load_w("cwk8", CC, H, FP8)
        cwv8 = load_w("cwv8", HC, C, FP8)
        cwr8 = load_w("cwr8", CC, C, FP8)

    def dr_fold(out_ps, wa, wb, xnB8, hh, th):
        """k/v/r: 4 DoubleRow matmuls, contraction 1024 = (aligned 512 +
        shifted 512), accumulating into out_ps [128, 512]."""
        t0 = 32 + th * 512
        for j in range(2):
            nc.tensor.matmul(out_ps[:], wa[:, 2 * j:2 * j + 2,
                                           hh * 128:(hh + 1) * 128],
                             xnB8[:, 2 * j:2 * j + 2, t0:t0 + 512],
                             start=(j == 0), stop=False, perf_mode=DR)
        for j in range(2):
            nc.tensor.matmul(out_ps[:], wb[:, 2 * j:2 * j + 2,
                                           hh * 128:(hh + 1) * 128],
                             xnB8[:, 2 * j:2 * j + 2, t0 - 1:t0 + 511],
                             start=False, stop=(j == 1), perf_mode=DR)

    def stage_K(b):
        """k/v/r matmuls + WKV chains -> srw(b) [128, CC, T] fp8."""
        xnB8 = xnB8_t[b]
        srw = srwp.tile([128, CC, T], FP8, tag="srw", name=f"srw{b}")
        for hh in range(CC):
            k_ps = ps2.tile([128, 1024], F32, tag="ps2", name=f"kps{b}_{hh}")
            for th in range(2):
                dr_fold(k_ps[:, th * 512:(th + 1) * 512], wka, wkb,
                        xnB8, hh, th)
            e_t = wkvp.tile([128, T], BF16, tag="e")
            nc.scalar.activation(e_t[:], k_ps[:], AF.Exp, scale=ISC,
                                 bias=nln64_t[:])
            r_ps = ps2.tile([128, 1024], F32, tag="ps2", name=f"rps{b}_{hh}")
            for th in range(2):
                dr_fold(r_ps[:, th * 512:(th + 1) * 512], wra, wrb,
                        xnB8, hh, th)
            sig = wkvp.tile([128, T], BF16, tag="sig")
            nc.scalar.activation(sig[:], r_ps[:], AF.Sigmoid, scale=ISC,
                                 bias=rb_c[hh][:])
            v_ps = ps2.tile([128, 1024], F32, tag="ps2", name=f"vps{b}_{hh}")
            for th in range(2):
                dr_fold(v_ps[:, th * 512:(th + 1) * 512], wva, wvb,
                        xnB8, hh, th)
            vcp = wkvp.tile([128, T], BF16, tag="vcp")
            nc.scalar.activation(vcp[:], v_ps[:], AF.Copy)
            ev = wkvp.tile([128, T], BF16, tag="ev")
            nc.gpsimd.tensor_tensor(ev[:], e_t[:], vcp[:], op=OP.mult)
            Pb = wkvp.tile([128, T + 1], BF16, tag="Pb")
            Qb = wkvp.tile([128, T + 1], F32, tag="Qb")
            nc.vector.memset(Pb[:, 0:1], 0.0)
            nc.vector.memset(Qb[:, 0:1], 0.0)
            db = delta_c[hh][:].to_broadcast((128, T))
            nc.vector.tensor_tensor_scan(Pb[:, 1:T + 1], db, ev[:],
                                         0.0, op0=OP.mult, op1=OP.add)
            nc.vector.tensor_tensor_scan(Qb[:, 1:T + 1], db, e_t[:],
                                         0.0, op0=OP.mult, op1=OP.add)
            # N' into ev, D' into Qb (slot t holds Q'_{t-1}), both in place
            nc.vector.scalar_tensor_tensor(ev[:], ev[:], eu_c[hh][:],
                                           Pb[:, 0:T], op0=OP.mult, op1=OP.add)
            nc.vector.scalar_tensor_tensor(Qb[:, 0:T], e_t[:], eu_c[hh][:],
                                           Qb[:, 0:T], op0=OP.mult, op1=OP.add)
            df = wkvp.tile([128, T], F32, tag="df")
            nc.vector.reciprocal_approx_fast(df[:], Qb[:, 0:T])
            nc.vector.scalar_tensor_tensor(ev[:], ev[:], 1.0, df[:],
                                           op0=OP.bypass, op1=OP.mult)
            nc.gpsimd.tensor_tensor(srw[:, hh, :], ev[:], sig[:], op=OP.mult)
        srw_t[b] = srw

    def stage_W(b):
        """Wo (fp8 DR, srw-stationary) + residual add in place: xa -> x1."""
        xa = xa_t[b]
        srw = srw_t[b]
        for n in range(NT):
            p_ = ps.tile([128, 512], F32, tag="ps", name=f"wops{b}_{n}")
            for j in range(2):
                nc.tensor.matmul(p_[:],
                                 srw[:, 2 * j:2 * j + 2,
                                     n * 128:(n + 1) * 128],
                                 woT8[:, 2 * j:2 * j + 2, :],
                                 start=(j == 0), stop=False, perf_mode=DR)
            # += 512*xa via identity matmul, then x1 = p/512 on ACT
            nc.tensor.matmul(p_[:], ident[:], xa[:, n, :],
                             start=False, stop=True)
            nc.scalar.activation(xa[:, n, :], p_[:], AF.Copy,
                                 scale=1.0 / 512.0)

    xn2B_t = {}

    def stage_L(b):
        """LN2 + xk2 mix (fp8); xr2 deferred to stage_L2."""
        xn2B = layer_norm(b, xa_t[b], 2)
        xn2B_t[b] = xn2B
        d2 = b2p.tile([128, CC, T], BF16, tag="d2", name=f"d2_{b}")
        xk2 = b2p.tile([128, CC, T], FP8, tag="xk2", name=f"xk2_{b}")
        for cc in range(CC):
            nc.gpsimd.tensor_tensor(d2[:, cc, :], xn2B[:, cc, 32:T + 32],
                                    xn2B[:, cc, 31:T + 31], op=OP.subtract)
        for cc in range(CC):
            nc.vector.scalar_tensor_tensor(xk2[:, cc, :], d2[:, cc, :],
                                           cmk_c[cc][:], xn2B[:, cc, 32:T + 32],
                                           op0=OP.mult, op1=OP.add)
        xk2_t[b] = xk2
        return d2

    def stage_L2(b, d2):
        xn2B = xn2B_t[b]
        xr2 = b2p.tile([128, CC, T], FP8, tag="xr2", name=f"xr2_{b}")
        for cc in range(CC):
            nc.vector.scalar_tensor_tensor(xr2[:, cc, :], d2[:, cc, :],
                                           cmr_c[cc][:], xn2B[:, cc, 32:T + 32],
                                           op0=OP.mult, op1=OP.add)
        xr2_t[b] = xr2

    def stage_M(b):
        """Channel mix: kk = relu(cWk xk2)^2 (fp8), rkv, residual, store."""
        xk2, xr2 = xk2_t[b], xr2_t[b]
        x1 = xa_t[b]
        yb = y_d[b].rearrange("(n p) c -> p n c", p=128)
        kk2 = cmp_.tile([128, HC, T], FP8, tag="kk2", name=f"kk2_{b}")
        rl = cmp_.tile([128, 2, T], BF16, tag="rl")
        for g in range(HC // 2):
            pps = []
            for u in range(2):
                hh = 2 * g + u
                p_ = ps2.tile([128, 1024], F32, tag="ps2",
                              name=f"kkps{b}_{hh}")
                for th in range(2):
                    for j in range(2):
                        nc.tensor.matmul(
                            p_[:, th * 512:(th + 1) * 512],
                            cwk8[:, 2 * j:2 * j + 2, hh * 128:(hh + 1) * 128],
                            xk2[:, 2 * j:2 * j + 2, th * 512:(th + 1) * 512],
                            start=(j == 0), stop=(j == 1), perf_mode=DR)
                pps.append((hh, p_))
            for u, (hh, p_) in enumerate(pps):
                nc.scalar.activation(rl[:, u, :], p_[:], AF.Relu, scale=ISC,
                                     bias=kkb_c[hh][:])
            for u, (hh, p_) in enumerate(pps):
                nc.scalar.activation(kk2[:, hh, :], rl[:, u, :], AF.Square)
        for n in range(NT):
            rp = ps.tile([128, 512], F32, tag="ps", name=f"rp{b}_{n}")
            for j in range(2):
                nc.tensor.matmul(rp[:],
                                 xr2[:, 2 * j:2 * j + 2,
                                     n * 128:(n + 1) * 128],
                                 cwr8[:, 2 * j:2 * j + 2, :],
                                 start=(j == 0), stop=(j == 1), perf_mode=DR)
            sig2 = outp.tile([128, 512], BF16, tag="sig2")
            nc.scalar.activation(sig2[:], rp[:], AF.Sigmoid, scale=ISC)
            kvp = ps.tile([128, 512], F32, tag="ps", name=f"kvp{b}_{n}")
            for j in range(HC // 2):
                nc.tensor.matmul(
                    kvp[:], kk2[:, 2 * j:2 * j + 2, n * 128:(n + 1) * 128],
                    cwv8[:, 2 * j:2 * j + 2, :],
                    start=(j == 0), stop=(j == HC // 2 - 1), perf_mode=DR)
            kvs = outp.tile([128, 512], F32, tag="kvs")
            nc.vector.tensor_tensor(kvs[:], kvp[:], sig2[:], op=OP.mult)
            nc.vector.scalar_tensor_tensor(kvs[:], kvs[:], ISC, x1[:, n, :],
                                           op0=OP.mult, op1=OP.add)
            nc.gpsimd.dma_start(yb[:, n, :], kvs[:])

    # ---- software pipeline over batches ----
    # PE queue order: K(0) | W(0) K(1) M(0) | W(1) K(2) M(1) | ... | M(bl-1)
    stage_A(0)
    stage_weights_tm()
    stage_weights_cm()
    stage_K(0)
    if bl > 1:
        stage_A(1)
    for b in range(bl):
        stage_W(b)
        d2 = stage_L(b)
        if b + 1 < bl:
            stage_K(b + 1)
        stage_L2(b, d2)
        if b + 2 < bl:
            stage_A(b + 2)
        stage_M(b)


def build_program(bl=BL):
    nc = bacc.Bacc("TRN2", target_bir_lowering=False, debug=False,
                   num_devices=NCORES)
    io = {}
    io["x"] = nc.dram_tensor("x", [bl, T, C], F32, kind="ExternalInput")
    io["y"] = nc.dram_tensor("y", [bl, T, C], F32, kind="ExternalOutput")
    for nm, d1, d2 in [("wka", CC, C), ("wkb", CC, C), ("wva", CC, C),
                       ("wvb", CC, C), ("wra", CC, C), ("wrb", CC, C),
                       ("woT8", CC, C), ("cwk8", CC, H), ("cwv8", HC, C),
                       ("cwr8", CC, C)]:
        io[nm] = nc.dram_tensor(nm, [128, d1, d2], FP8, kind="ExternalInput")
    for nm, n in [("delta", C), ("eu", C), ("rb", C),
                  ("cmk", C), ("cmr", C), ("kkb", H)]:
        io[nm] = nc.dram_tensor(nm, [n], F32, kind="ExternalInput")
    io["ident512"] = nc.dram_tensor("ident512", [128, 128], F32,
                                    kind="ExternalInput")

    with tile.TileContext(nc) as tc:
        with ExitStack() as ctx:
            _emit(nc, tc, ctx, io, bl)
    nc.compile()
    return nc


def _pack8(w, scale=SC):  # [C_in, M] f32 -> [128, C_in//128, M] fp8e4m3
    ci, m = w.shape
    w8 = np.clip(w * scale, -240.0, 240.0).astype(ml_dtypes.float8_e4m3)
    return np.ascontiguousarray(w8.reshape(ci // 128, 128, m).transpose(1, 0, 2))


def host_params(inputs):
    """Host-side parameter prep (O(C^2) only)."""
    f32 = np.float32
    g1 = np.asarray(inputs["ln1_g"], f32)
    b1 = np.asarray(inputs["ln1_b"], f32)
    g2 = np.asarray(inputs["ln2_g"], f32)
    b2 = np.asarray(inputs["ln2_b"], f32)
    Wk = np.asarray(inputs["Wk"], f32)
    Wv = np.asarray(inputs["Wv"], f32)
    Wr = np.asarray(inputs["Wr"], f32)
    Wo = np.asarray(inputs["Wo"], f32)
    cWk = np.asarray(inputs["cWk"], f32)
    cWr = np.asarray(inputs["cWr"], f32)
    cWv = np.asarray(inputs["cWv"], f32)
    tmk = np.asarray(inputs["tm_k"], f32)
    tmv = np.asarray(inputs["tm_v"], f32)
    tmr = np.asarray(inputs["tm_r"], f32)

    # biases from LN betas land inside ACT bias terms; only the zero case is
    # supported (true for this model's init)
    assert np.allclose(Wk @ b1, 0.0, atol=1e-30), "nonzero ln1_b unsupported"
    assert np.allclose(Wv @ b1, 0.0, atol=1e-30), "nonzero ln1_b unsupported"
    assert np.allclose(cWr @ b2, 0.0, atol=1e-30), "nonzero ln2_b unsupported"

    p = {
        "wka": _pack8(Wk.T * (g1 * tmk)[:, None]),
        "wkb": _pack8(Wk.T * (g1 * (1.0 - tmk))[:, None]),
        "wva": _pack8(Wv.T * (g1 * tmv)[:, None], SCV),
        "wvb": _pack8(Wv.T * (g1 * (1.0 - tmv))[:, None], SCV),
        "wra": _pack8(Wr.T * (g1 * tmr)[:, None]),
        "wrb": _pack8(Wr.T * (g1 * (1.0 - tmr))[:, None]),
        "woT8": _pack8(Wo.T),
        "cwk8": _pack8(cWk.T * g2[:, None]),
        "cwv8": _pack8(cWv.T),
        "cwr8": _pack8(cWr.T * g2[:, None]),
        "delta": np.exp(-np.exp(np.asarray(inputs["time_decay"], f32))),
        "eu": np.exp(np.asarray(inputs["time_first"], f32)),
        "cmk": np.asarray(inputs["cm_k"], f32),
        "cmr": np.asarray(inputs["cm_r"], f32),
        "rb": (Wr @ b1).astype(f32),
        "kkb": (cWk @ b2).astype(f32),
        "ident512": (np.eye(128, dtype=f32) * 512.0),
    }
    return p


_CACHE = {}


def kernel(**inputs):
    from concourse.bass_utils import run_bass_kernel_spmd

    if "nc" not in _CACHE:
        _CACHE["nc"] = build_program(BL)
    nc = _CACHE["nc"]

    p = host_params(inputs)
    x = np.asarray(inputs["x"], np.float32)
    in_maps = []
    for c in range(NCORES):
        m = dict(p)
        m["x"] = np.ascontiguousarray(x[c * BL:(c + 1) * BL])
        in_maps.append(m)
    res = run_bass_kernel_spmd(nc, in_maps, list(range(NCORES)))
    out = np.concatenate([res.results[c]["y"] for c in range(NCORES)], axis=0)
    return out.astype(np.float32)


# revision 37
# speedup vs baseline: 1.6432x; 1.0243x over previous
"""RWKV-v4 block (time-mix WKV attention + channel-mix GLU) on 8 Trainium2
NeuronCores, data-parallel over batch B.  v3: all matmuls fp8e4m3 DoubleRow
(2 contraction chunks per instruction, ~2x bf16 column rate), bf16 WKV chain
in STT form on DVE, software-pipelined across the 4 local batches.

Layouts per core (B_local=4, T=1024, C=512, H=2048):
  - layout A: [t(128p), n(8), c(512)] -- LayerNorm (bn_stats), residual adds,
    final store.
  - layout B: [c(128p), cc(4), t(1056)] -- WKV scan along free dim, matmul
    operands.  A->B via bf16 DMA transpose through a DRAM bounce (32 zero
    cols in front make the token shift an offset view), then fp8 quantize.

Weight scaling (fp8 weights packed x64 except Wv x8):
  k_raw = 64k   -> e  = exp(k_raw/64 - ln64) = e_true/64        (ACT bias)
  v_raw = 8v    -> ev = e*v_raw = e_true*v/8
  P' = P/8 (scan ev), Q' = Q/64 (scan e)
  N' = eu*ev + P'_{t-1} = N/8,  D' = eu*e + Q'_{t-1} = D/64
  y' = N'/D' = 8y  -> srw = y'*sig in fp8 (|8y*sig| < 40 ok)
  att_raw = (64*Wo)@srw = 512*att -> x1 = xa + att_raw/512      (STT)
  r_raw = 64r -> sigmoid(r_raw/64 + rb); cWk/cWr/cWv x64 likewise.
The time-mix token shift is folded into k/v/r weights (Wa = W*diag(g*tm),
Wb = W*diag(g*(1-tm))), the shifted moving operand is an offset view of the
same fp8 tile.  Channel-mix shifts stay explicit (output dim >> contraction).
"""

import numpy as np
import ml_dtypes
from contextlib import ExitStack

import concourse.bass as bass
import concourse.tile as tile
from concourse import bacc, mybir

B, T, C = 32, 1024, 512
H = 4 * C
NCORES = 8
BL = B // NCORES  # batches per core
NT = T // 128     # 8 t-subtiles per batch
CC = C // 128     # 4 channel chunks
HC = H // 128     # 16 hidden chunks
SC = 64.0         # fp8 weight scale
ISC = 1.0 / SC
SCV = 8.0         # Wv fp8 scale
LN64 = float(np.log(64.0))

F32 = mybir.dt.float32
BF16 = mybir.dt.bfloat16
FP8 = mybir.dt.float8e4
AX = mybir.AxisListType
OP = mybir.AluOpType
AF = mybir.ActivationFunctionType
DR = mybir.MatmulPerfMode.DoubleRow


def _emit(nc, tc, ctx, io, bl):
    x_d = io["x"].ap()
    y_d = io["y"].ap()

    def col(name, c0):  # [128,1] slice of a [N] dram vector
        return io[name].ap()[c0 * 128:(c0 + 1) * 128].rearrange(
            "(c one) -> c one", one=1)

    sb = ctx.enter_context(tc.tile_pool(name="sb", bufs=1))
    ps2 = ctx.enter_context(tc.tile_pool(name="ps2", bufs=3, space="PSUM"))
    ps = ctx.enter_context(tc.tile_pool(name="ps", bufs=2, space="PSUM"))
    dramp = ctx.enter_context(tc.tile_pool(name="dram", bufs=4, space="DRAM"))

    # ---- small consts first (cheap), then x(b0) so LN1 starts immediately;
    # weights stream in behind it in first-use order ----
    def vecload(name, n=CC, madd=None):
        ts_ = []
        for i in range(n):
            t_ = sb.tile([128, 1], F32, tag=f"v_{name}_{i}")
            nc.gpsimd.dma_start(t_[:], col(name, i))
            if madd is not None:
                nc.vector.tensor_scalar_add(t_[:], t_[:], madd)
            ts_.append(t_)
        return ts_

    eps_t = sb.tile([128, 1], F32, tag="eps")
    nc.vector.memset(eps_t[:], 1e-5)
    nln64_t = sb.tile([128, 1], F32, tag="nln64")
    nc.vector.memset(nln64_t[:], -LN64)
    ident = sb.tile([128, 128], F32, tag="ident")
    nc.gpsimd.dma_start(ident[:], io["ident512"].ap())
    zrow = sb.tile([32, C], BF16, tag="zrow")
    nc.vector.memset(zrow[:], 0.0)

    delta_c = vecload("delta")
    eu_c = vecload("eu")
    rb_c = vecload("rb")
    kkb_c = vecload("kkb", HC)
    cmk_c = vecload("cmk", madd=-1.0)
    cmr_c = vecload("cmr", madd=-1.0)

    # ---- per-batch pools ----
    xa_pool = ctx.enter_context(tc.tile_pool(name="xa", bufs=2))
    lnp = ctx.enter_context(tc.tile_pool(name="ln", bufs=1))
    bq = ctx.enter_context(tc.tile_pool(name="bq", bufs=1))
    bp = ctx.enter_context(tc.tile_pool(name="bp", bufs=2))
    b2p = ctx.enter_context(tc.tile_pool(name="b2p", bufs=1))
    wkvp = ctx.enter_context(tc.tile_pool(name="wkv", bufs=2))
    srwp = ctx.enter_context(tc.tile_pool(name="srw", bufs=1))
    cmp_ = ctx.enter_context(tc.tile_pool(name="cm", bufs=1))
    outp = ctx.enter_context(tc.tile_pool(name="out", bufs=2))

    xa_t = {}     # b -> [128, NT, 512] f32 (becomes x1 in place after Wo)
    xnB8_t = {}   # b -> [128, CC, 1056] fp8 (LN1, k/v/r folded matmuls)
    xk2_t = {}    # b -> [128, CC, T] fp8
    xr2_t = {}    # b -> [128, CC, T] fp8
    srw_t = {}    # b -> [128, CC, T] fp8

    def layer_norm(b, src_tile, which):
        """[128, NT, 512] f32 layout A -> xnB bf16 [128, CC, 1056] layout B
        (32 zero cols in front for the token shift)."""
        bnst = lnp.tile([128, NT, 6], F32, tag=f"bnst{which}")
        for n in range(NT):
            nc.vector.bn_stats(bnst[:, n, :], src_tile[:, n, :])
        mv = lnp.tile([128, NT, 2], F32, tag=f"mv{which}")
        for n in range(NT):
            nc.vector.bn_aggr(mv[:, n, :], bnst[:, n, :])
        sqv = lnp.tile([128, NT], F32, tag=f"sqv{which}")
        nc.scalar.activation(sqv[:], mv[:, :, 1], AF.Sqrt, bias=eps_t[:])
        rstd = lnp.tile([128, NT], F32, tag=f"rstd{which}")
        nc.vector.reciprocal(rstd[:], sqv[:])
        xn = lnp.tile([128, NT, 512], BF16, tag=f"xn{which}")
        for n in range(NT):
            nc.vector.tensor_scalar(xn[:, n, :], src_tile[:, n, :],
                                    mv[:, n, 0:1], rstd[:, n:n + 1],
                                    op0=OP.subtract, op1=OP.mult)
        xnd = dramp.tile([T + 32, C], BF16, tag=f"xnd{which}")
        nc.sync.dma_start(xnd[0:32, :], zrow[:])
        nc.sync.dma_start(xnd[32:T + 32].rearrange("(n p) c -> p n c", p=128),
                          xn[:])
        pool = bq if which == 1 else b2p
        xnB = pool.tile([128, CC, T + 32], BF16, tag=f"xnB{which}")
        for cc in range(CC):
            nc.sync.dma_start_transpose(xnB[:, cc, :],
                                        xnd[:, cc * 128:(cc + 1) * 128])
        return xnB

    def stage_A(b):
        """load x(b), LN1(b), fp8 quantize."""
        xb = x_d[b].rearrange("(n p) c -> p n c", p=128)
        xa = xa_pool.tile([128, NT, 512], F32, tag="xa", name=f"xa{b}")
        nc.sync.dma_start(xa[:], xb)
        xa_t[b] = xa
        xnB = layer_norm(b, xa, 1)
        xnB8 = bp.tile([128, CC, T + 32], FP8, tag="xnB8", name=f"xnB8_{b}")
        for cc in range(CC):
            nc.scalar.activation(xnB8[:, cc, :], xnB[:, cc, :], AF.Copy)
        xnB8_t[b] = xnB8

    # ---- weights (resident), loaded after batch-0's x ----
    def load_w(name, d1, d2, dt):
        t_ = sb.tile([128, d1, d2], dt, tag=f"w_{name}")
        nc.gpsimd.dma_start(t_[:], io[name].ap())
        return t_

    def stage_weights_tm():
        global wka, wkb, wva, wvb, wra, wrb, woT8
        wka = load_w("wka", CC, C, FP8)
        wkb = load_w("wkb", CC, C, FP8)
        wva = load_w("wva", CC, C, FP8)
        wvb = load_w("wvb", CC, C, FP8)
        wra = load_w("wra", CC, C, FP8)
        wrb = load_w("wrb", CC, C, FP8)
        woT8 = load_w("woT8", CC, C, FP8)

    def stage_weights_cm():
        global cwk8, cwv8, cwr8
        cwk8 = load_w("cwk8", CC, H, FP8)
        cwv8 = load_w("cwv8", HC, C, FP8)
        cwr8 = load_w("cwr8", CC, C, FP8)

    def dr_fold(out_ps, wa, wb, xnB8, hh, th):
        """k/v/r: 4 DoubleRow matmuls, contraction 1024 = (aligned 512 +
        shifted 512), accumulating into out_ps [128, 512]."""
        t0 = 32 + th * 512
        for j in range(2):
            nc.tensor.matmul(out_ps[:], wa[:, 2 * j:2 * j + 2,
                                           hh * 128:(hh + 1) * 128],
                             xnB8[:, 2 * j:2 * j + 2, t0:t0 + 512],
                             start=(j == 0), stop=False, perf_mode=DR)
        for j in range(2):
            nc.tensor.matmul(out_ps[:], wb[:, 2 * j:2 * j + 2,
                                           hh * 128:(hh + 1) * 128],
                             xnB8[:, 2 * j:2 * j + 2, t0 - 1:t0 + 511],
                             start=False, stop=(j == 1), perf_mode=DR)

    def stage_K(b):
        """k/v/r matmuls + WKV chains -> srw(b) [128, CC, T] fp8."""
        xnB8 = xnB8_t[b]
        srw = srwp.tile([128, CC, T], FP8, tag="srw", name=f"srw{b}")
        for hh in range(CC):
            k_ps = ps2.tile([128, 1024], F32, tag="ps2", name=f"kps{b}_{hh}")
            for th in range(2):
                dr_fold(k_ps[:, th * 512:(th + 1) * 512], wka, wkb,
                        xnB8, hh, th)
            e_t = wkvp.tile([128, T], BF16, tag="e")
            nc.scalar.activation(e_t[:], k_ps[:], AF.Exp, scale=ISC,
                                 bias=nln64_t[:])
            r_ps = ps2.tile([128, 1024], F32, tag="ps2", name=f"rps{b}_{hh}")
            for th in range(2):
                dr_fold(r_ps[:, th * 512:(th + 1) * 512], wra, wrb,
                        xnB8, hh, th)
            sig = wkvp.tile([128, T], BF16, tag="sig")
            nc.scalar.activation(sig[:], r_ps[:], AF.Sigmoid, scale=ISC,
                                 bias=rb_c[hh][:])
            v_ps = ps2.tile([128, 1024], F32, tag="ps2", name=f"vps{b}_{hh}")
            for th in range(2):
                dr_fold(v_ps[:, th * 512:(th + 1) * 512], wva, wvb,
                        xnB8, hh, th)
            vcp = wkvp.tile([128, T], BF16, tag="vcp")
            nc.scalar.activation(vcp[:], v_ps[:], AF.Copy)
            ev = wkvp.tile([128, T], BF16, tag="ev")
            nc.gpsimd.tensor_tensor(ev[:], e_t[:], vcp[:], op=OP.mult)
            Pb = wkvp.tile([128, T + 1], BF16, tag="Pb")
            Qb = wkvp.tile([128, T + 1], F32, tag="Qb")
            nc.vector.memset(Pb[:, 0:1], 0.0)
            nc.vector.memset(Qb[:, 0:1], 0.0)
            db = delta_c[hh][:].to_broadcast((128, T))
            nc.vector.tensor_tensor_scan(Pb[:, 1:T + 1], db, ev[:],
                                         0.0, op0=OP.mult, op1=OP.add)
            nc.vector.tensor_tensor_scan(Qb[:, 1:T + 1], db, e_t[:],
                                         0.0, op0=OP.mult, op1=OP.add)
            # N' into ev, D' into Qb (slot t holds Q'_{t-1}), both in place
            nc.vector.scalar_tensor_tensor(ev[:], ev[:], eu_c[hh][:],
                                           Pb[:, 0:T], op0=OP.mult, op1=OP.add)
            nc.vector.scalar_tensor_tensor(Qb[:, 0:T], e_t[:], eu_c[hh][:],
                                           Qb[:, 0:T], op0=OP.mult, op1=OP.add)
            df = wkvp.tile([128, T], F32, tag="df")
            nc.vector.reciprocal_approx_fast(df[:], Qb[:, 0:T])
            nc.vector.scalar_tensor_tensor(ev[:], ev[:], 1.0, df[:],
                                           op0=OP.bypass, op1=OP.mult)
            nc.gpsimd.tensor_tensor(srw[:, hh, :], ev[:], sig[:], op=OP.mult)
        srw_t[b] = srw

    def stage_W(b):
        """Wo (fp8 DR, srw-stationary) + residual add in place: xa -> x1."""
        xa = xa_t[b]
        srw = srw_t[b]
        for n in range(NT):
            p_ = ps.tile([128, 512], F32, tag="ps", name=f"wops{b}_{n}")
            for j in range(2):
                nc.tensor.matmul(p_[:],
                                 srw[:, 2 * j:2 * j + 2,
                                     n * 128:(n + 1) * 128],
                                 woT8[:, 2 * j:2 * j + 2, :],
                                 start=(j == 0), stop=(j == 1), perf_mode=DR)
            nc.vector.scalar_tensor_tensor(xa[:, n, :], p_[:], 1.0 / 512.0,
                                           xa[:, n, :],
                                           op0=OP.mult, op1=OP.add)

    xn2B_t = {}

    def stage_L(b):
        """LN2 + xk2 mix (fp8); xr2 deferred to stage_L2."""
        xn2B = layer_norm(b, xa_t[b], 2)
        xn2B_t[b] = xn2B
        d2 = b2p.tile([128, CC, T], BF16, tag="d2", name=f"d2_{b}")
        xk2 = b2p.tile([128, CC, T], FP8, tag="xk2", name=f"xk2_{b}")
        for cc in range(CC):
            nc.gpsimd.tensor_tensor(d2[:, cc, :], xn2B[:, cc, 32:T + 32],
                                    xn2B[:, cc, 31:T + 31], op=OP.subtract)
        for cc in range(CC):
            nc.vector.scalar_tensor_tensor(xk2[:, cc, :], d2[:, cc, :],
                                           cmk_c[cc][:], xn2B[:, cc, 32:T + 32],
                                           op0=OP.mult, op1=OP.add)
        xk2_t[b] = xk2
        return d2

    def stage_L2(b, d2):
        xn2B = xn2B_t[b]
        xr2 = b2p.tile([128, CC, T], FP8, tag="xr2", name=f"xr2_{b}")
        for cc in range(CC):
            nc.vector.scalar_tensor_tensor(xr2[:, cc, :], d2[:, cc, :],
                                           cmr_c[cc][:], xn2B[:, cc, 32:T + 32],
                                           op0=OP.mult, op1=OP.add)
        xr2_t[b] = xr2

    def stage_M(b):
        """Channel mix: kk = relu(cWk xk2)^2 (fp8), rkv, residual, store."""
        xk2, xr2 = xk2_t[b], xr2_t[b]
        x1 = xa_t[b]
        yb = y_d[b].rearrange("(n p) c -> p n c", p=128)
        kk2 = cmp_.tile([128, HC, T], FP8, tag="kk2", name=f"kk2_{b}")
        rl = cmp_.tile([128, 2, T], BF16, tag="rl")
        for g in range(HC // 2):
            pps = []
            for u in range(2):
                hh = 2 * g + u
                p_ = ps2.tile([128, 1024], F32, tag="ps2",
                              name=f"kkps{b}_{hh}")
                for th in range(2):
                    for j in range(2):
                        nc.tensor.matmul(
                            p_[:, th * 512:(th + 1) * 512],
                            cwk8[:, 2 * j:2 * j + 2, hh * 128:(hh + 1) * 128],
                            xk2[:, 2 * j:2 * j + 2, th * 512:(th + 1) * 512],
                            start=(j == 0), stop=(j == 1), perf_mode=DR)
                pps.append((hh, p_))
            for u, (hh, p_) in enumerate(pps):
                nc.scalar.activation(rl[:, u, :], p_[:], AF.Relu, scale=ISC,
                                     bias=kkb_c[hh][:])
            for u, (hh, p_) in enumerate(pps):
                nc.scalar.activation(kk2[:, hh, :], rl[:, u, :], AF.Square)
        for n in range(NT):
            rp = ps.tile([128, 512], F32, tag="ps", name=f"rp{b}_{n}")
            for j in range(2):
                nc.tensor.matmul(rp[:],
                                 xr2[:, 2 * j:2 * j + 2,
                                     n * 128:(n + 1) * 128],
                                 cwr8[:, 2 * j:2 * j + 2, :],
                                 start=(j == 0), stop=(j == 1), perf_mode=DR)
            sig2 = outp.tile([128, 512], BF16, tag="sig2")
            nc.scalar.activation(sig2[:], rp[:], AF.Sigmoid, scale=ISC)
            kvp = ps.tile([128, 512], F32, tag="ps", name=f"kvp{b}_{n}")
            for j in range(HC // 2):
                nc.tensor.matmul(
                    kvp[:], kk2[:, 2 * j:2 * j + 2, n * 128:(n + 1) * 128],
                    cwv8[:, 2 * j:2 * j + 2, :],
                    start=(j == 0), stop=(j == HC // 2 - 1), perf_mode=DR)
            kvs = outp.tile([128, 512], F32, tag="kvs")
            nc.vector.tensor_tensor(kvs[:], kvp[:], sig2[:], op=OP.mult)
            nc.vector.scalar_tensor_tensor(kvs[:], kvs[:], ISC, x1[:, n, :],
                                           op0=OP.mult, op1=OP.add)
            nc.gpsimd.dma_start(yb[:, n, :], kvs[:])

    # ---- software pipeline over batches ----
    # PE queue order: K(0) | W(0) K(1) M(0) | W(1) K(2) M(1) | ... | M(bl-1)
    stage_A(0)
    stage_weights_tm()
    stage_weights_cm()
    stage_K(0)
    if bl > 1:
        stage_A(1)
    for b in range(bl):
        stage_W(b)
        d2 = stage_L(b)
        stage_L2(b, d2)
        if b + 1 < bl:
            stage_K(b + 1)
        if b + 2 < bl:
            stage_A(b + 2)
        stage_M(b)


def build_program(bl=BL):
    nc = bacc.Bacc("TRN2", target_bir_lowering=False, debug=False,
                   num_devices=NCORES)
    io = {}
    io["x"] = nc.dram_tensor("x", [bl, T, C], F32, kind="ExternalInput")
    io["y"] = nc.dram_tensor("y", [bl, T, C], F32, kind="ExternalOutput")
    for nm, d1, d2 in [("wka", CC, C), ("wkb", CC, C), ("wva", CC, C),
                       ("wvb", CC, C), ("wra", CC, C), ("wrb", CC, C),
                       ("woT8", CC, C), ("cwk8", CC, H), ("cwv8", HC, C),
                       ("cwr8", CC, C)]:
        io[nm] = nc.dram_tensor(nm, [128, d1, d2], FP8, kind="ExternalInput")
    for nm, n in [("delta", C), ("eu", C), ("rb", C),
                  ("cmk", C), ("cmr", C), ("kkb", H)]:
        io[nm] = nc.dram_tensor(nm, [n], F32, kind="ExternalInput")
    io["ident512"] = nc.dram_tensor("ident512", [128, 128], F32,
                                    kind="ExternalInput")

    with tile.TileContext(nc) as tc:
        with ExitStack() as ctx:
            _emit(nc, tc, ctx, io, bl)
    nc.compile()
    return nc


def _pack8(w, scale=SC):  # [C_in, M] f32 -> [128, C_in//128, M] fp8e4m3
    ci, m = w.shape
    w8 = np.clip(w * scale, -240.0, 240.0).astype(ml_dtypes.float8_e4m3)
    return np.ascontiguousarray(w8.reshape(ci // 128, 128, m).transpose(1, 0, 2))


def host_params(inputs):
    """Host-side parameter prep (O(C^2) only)."""
    f32 = np.float32
    g1 = np.asarray(inputs["ln1_g"], f32)
    b1 = np.asarray(inputs["ln1_b"], f32)
    g2 = np.asarray(inputs["ln2_g"], f32)
    b2 = np.asarray(inputs["ln2_b"], f32)
    Wk = np.asarray(inputs["Wk"], f32)
    Wv = np.asarray(inputs["Wv"], f32)
    Wr = np.asarray(inputs["Wr"], f32)
    Wo = np.asarray(inputs["Wo"], f32)
    cWk = np.asarray(inputs["cWk"], f32)
    cWr = np.asarray(inputs["cWr"], f32)
    cWv = np.asarray(inputs["cWv"], f32)
    tmk = np.asarray(inputs["tm_k"], f32)
    tmv = np.asarray(inputs["tm_v"], f32)
    tmr = np.asarray(inputs["tm_r"], f32)

    # biases from LN betas land inside ACT bias terms; only the zero case is
    # supported (true for this model's init)
    assert np.allclose(Wk @ b1, 0.0, atol=1e-30), "nonzero ln1_b unsupported"
    assert np.allclose(Wv @ b1, 0.0, atol=1e-30), "nonzero ln1_b unsupported"
    assert np.allclose(cWr @ b2, 0.0, atol=1e-30), "nonzero ln2_b unsupported"

    p = {
        "wka": _pack8(Wk.T * (g1 * tmk)[:, None]),
        "wkb": _pack8(Wk.T * (g1 * (1.0 - tmk))[:, None]),
        "wva": _pack8(Wv.T * (g1 * tmv)[:, None], SCV),
        "wvb": _pack8(Wv.T * (g1 * (1.0 - tmv))[:, None], SCV),
        "wra": _pack8(Wr.T * (g1 * tmr)[:, None]),
        "wrb": _pack8(Wr.T * (g1 * (1.0 - tmr))[:, None]),
        "woT8": _pack8(Wo.T),
        "cwk8": _pack8(cWk.T * g2[:, None]),
        "cwv8": _pack8(cWv.T),
        "cwr8": _pack8(cWr.T * g2[:, None]),
        "delta": np.exp(-np.exp(np.asarray(inputs["time_decay"], f32))),
        "eu": np.exp(np.asarray(inputs["time_first"], f32)),
        "cmk": np.asarray(inputs["cm_k"], f32),
        "cmr": np.asarray(inputs["cm_r"], f32),
        "rb": (Wr @ b1).astype(f32),
        "kkb": (cWk @ b2).astype(f32),
        "ident512": (np.eye(128, dtype=f32) * 512.0),
    }
    return p


_CACHE = {}


def kernel(**inputs):
    from concourse.bass_utils import run_bass_kernel_spmd

    if "nc" not in _CACHE:
        _CACHE["nc"] = build_program(BL)
    nc = _CACHE["nc"]

    p = host_params(inputs)
    x = np.asarray(inputs["x"], np.float32)
    in_maps = []
    for c in range(NCORES):
        m = dict(p)
        m["x"] = np.ascontiguousarray(x[c * BL:(c + 1) * BL])
        in_maps.append(m)
    res = run_bass_kernel_spmd(nc, in_maps, list(range(NCORES)))
    out = np.concatenate([res.results[c]["y"] for c in range(NCORES)], axis=0)
    return out.astype(np.float32)


# revision 44
# speedup vs baseline: 1.7728x; 1.0789x over previous
"""RWKV-v4 block (time-mix WKV attention + channel-mix GLU) on 8 Trainium2
NeuronCores, data-parallel over batch B.  v3: all matmuls fp8e4m3 DoubleRow
(2 contraction chunks per instruction, ~2x bf16 column rate), bf16 WKV chain
in STT form on DVE, software-pipelined across the 4 local batches.

Layouts per core (B_local=4, T=1024, C=512, H=2048):
  - layout A: [t(128p), n(8), c(512)] -- LayerNorm (bn_stats), residual adds,
    final store.
  - layout B: [c(128p), cc(4), t(1056)] -- WKV scan along free dim, matmul
    operands.  A->B via bf16 DMA transpose through a DRAM bounce (32 zero
    cols in front make the token shift an offset view), then fp8 quantize.

Weight scaling (fp8 weights packed x64 except Wv x8):
  k_raw = 64k   -> e  = exp(k_raw/64 - ln64) = e_true/64        (ACT bias)
  v_raw = 8v    -> ev = e*v_raw = e_true*v/8
  P' = P/8 (scan ev), Q' = Q/64 (scan e)
  N' = eu*ev + P'_{t-1} = N/8,  D' = eu*e + Q'_{t-1} = D/64
  y' = N'/D' = 8y  -> srw = y'*sig in fp8 (|8y*sig| < 40 ok)
  att_raw = (64*Wo)@srw = 512*att -> x1 = xa + att_raw/512      (STT)
  r_raw = 64r -> sigmoid(r_raw/64 + rb); cWk/cWr/cWv x64 likewise.
The time-mix token shift is folded into k/v/r weights (Wa = W*diag(g*tm),
Wb = W*diag(g*(1-tm))), the shifted moving operand is an offset view of the
same fp8 tile.  Channel-mix shifts stay explicit (output dim >> contraction).
"""

import numpy as np
import ml_dtypes
from contextlib import ExitStack

import concourse.bass as bass
import concourse.tile as tile
from concourse import bacc, mybir

B, T, C = 32, 1024, 512
H = 4 * C
NCORES = 8
BL = B // NCORES  # batches per core
NT = T // 128     # 8 t-subtiles per batch
CC = C // 128     # 4 channel chunks
HC = H // 128     # 16 hidden chunks
SC = 64.0         # fp8 weight scale
ISC = 1.0 / SC
SCV = 8.0         # Wv fp8 scale
LN64 = float(np.log(64.0))

F32 = mybir.dt.float32
BF16 = mybir.dt.bfloat16
FP8 = mybir.dt.float8e4
AX = mybir.AxisListType
OP = mybir.AluOpType
AF = mybir.ActivationFunctionType
DR = mybir.MatmulPerfMode.DoubleRow


def _emit(nc, tc, ctx, io, bl):
    x_d = io["x"].ap()
    y_d = io["y"].ap()

    def col(name, c0):  # [128,1] slice of a [N] dram vector
        return io[name].ap()[c0 * 128:(c0 + 1) * 128].rearrange(
            "(c one) -> c one", one=1)

    sb = ctx.enter_context(tc.tile_pool(name="sb", bufs=1))
    ps2 = ctx.enter_context(tc.tile_pool(name="ps2", bufs=3, space="PSUM"))
    ps = ctx.enter_context(tc.tile_pool(name="ps", bufs=2, space="PSUM"))
    dramp = ctx.enter_context(tc.tile_pool(name="dram", bufs=4, space="DRAM"))

    # ---- small consts first (cheap), then x(b0) so LN1 starts immediately;
    # weights stream in behind it in first-use order ----
    def vecload(name, n=CC, madd=None):
        ts_ = []
        for i in range(n):
            t_ = sb.tile([128, 1], F32, tag=f"v_{name}_{i}")
            nc.gpsimd.dma_start(t_[:], col(name, i))
            if madd is not None:
                nc.vector.tensor_scalar_add(t_[:], t_[:], madd)
            ts_.append(t_)
        return ts_

    eps_t = sb.tile([128, 1], F32, tag="eps")
    nc.vector.memset(eps_t[:], 1e-5)
    nln64_t = sb.tile([128, 1], F32, tag="nln64")
    nc.vector.memset(nln64_t[:], -LN64)
    identT = sb.tile([128, 128], BF16, tag="identT")
    nc.gpsimd.dma_start(identT[:], io["identT"].ap())
    zrow = sb.tile([32, C], BF16, tag="zrow")
    nc.vector.memset(zrow[:], 0.0)

    delta_c = vecload("delta")
    eu_c = vecload("eu")
    rb_c = vecload("rb")
    kkb_c = vecload("kkb", HC)
    cmk_c = vecload("cmk", madd=-1.0)
    cmr_c = vecload("cmr", madd=-1.0)

    # ---- per-batch pools ----
    xa_pool = ctx.enter_context(tc.tile_pool(name="xa", bufs=2))
    lnp = ctx.enter_context(tc.tile_pool(name="ln", bufs=1))
    bq = ctx.enter_context(tc.tile_pool(name="bq", bufs=1))
    bp = ctx.enter_context(tc.tile_pool(name="bp", bufs=2))
    b2p = ctx.enter_context(tc.tile_pool(name="b2p", bufs=1))
    wkvp = ctx.enter_context(tc.tile_pool(name="wkv", bufs=2))
    srwp = ctx.enter_context(tc.tile_pool(name="srw", bufs=1))
    cmp_ = ctx.enter_context(tc.tile_pool(name="cm", bufs=1))
    outp = ctx.enter_context(tc.tile_pool(name="out", bufs=2))

    xa_t = {}     # b -> [128, NT, 512] f32 (becomes x1 in place after Wo)
    xnB8_t = {}   # b -> [128, CC, 1056] fp8 (LN1, k/v/r folded matmuls)
    xk2_t = {}    # b -> [128, CC, T] fp8
    xr2_t = {}    # b -> [128, CC, T] fp8
    srw_t = {}    # b -> [128, CC, T] fp8

    def layer_norm(b, src_tile, which):
        """[128, NT, 512] f32 layout A -> xnB bf16 [128, CC, 1056] layout B
        (32 zero cols in front for the token shift)."""
        bnst = lnp.tile([128, NT, 6], F32, tag=f"bnst{which}")
        for n in range(NT):
            nc.vector.bn_stats(bnst[:, n, :], src_tile[:, n, :])
        mv = lnp.tile([128, NT, 2], F32, tag=f"mv{which}")
        for n in range(NT):
            nc.vector.bn_aggr(mv[:, n, :], bnst[:, n, :])
        sqv = lnp.tile([128, NT], F32, tag=f"sqv{which}")
        nc.scalar.activation(sqv[:], mv[:, :, 1], AF.Sqrt, bias=eps_t[:])
        rstd = lnp.tile([128, NT], F32, tag=f"rstd{which}")
        nc.vector.reciprocal(rstd[:], sqv[:])
        xn = lnp.tile([128, NT, 512], BF16, tag=f"xn{which}")
        for n in range(NT):
            nc.vector.tensor_scalar(xn[:, n, :], src_tile[:, n, :],
                                    mv[:, n, 0:1], rstd[:, n:n + 1],
                                    op0=OP.subtract, op1=OP.mult)
        # A->B on the PE: 32 [128,128] transposes through PSUM; the drain
        # copy quantizes to fp8 for LN1.  32 zero cols in front keep the
        # token shift an offset view.
        pool = bp if which == 1 else b2p
        dt = FP8 if which == 1 else BF16
        xnB = pool.tile([128, CC, T + 32], dt, tag=f"xnB{which}",
                        name=f"xnB{which}_{b}")
        for cc in range(CC):
            nc.vector.memset(xnB[:, cc, 0:32], 0.0)
        for n in range(NT):
            p_ = ps.tile([128, 512], F32, tag="ps", name=f"tp{which}{b}_{n}")
            pb = p_[:].bitcast(BF16)
            for cc in range(CC):
                nc.tensor.transpose(pb[:, cc * 128:(cc + 1) * 128],
                                    xn[:, n, cc * 128:(cc + 1) * 128],
                                    identT[:])
            nc.scalar.activation(
                xnB[:, :, 32 + n * 128:32 + (n + 1) * 128],
                pb[:, 0:512].rearrange("p (c t) -> p c t", c=CC),
                AF.Copy)
        return xnB

    def stage_A(b):
        """load x(b), LN1(b), fp8 quantize."""
        xb = x_d[b].rearrange("(n p) c -> p n c", p=128)
        xa = xa_pool.tile([128, NT, 512], F32, tag="xa", name=f"xa{b}")
        nc.sync.dma_start(xa[:], xb)
        xa_t[b] = xa
        xnB8_t[b] = layer_norm(b, xa, 1)

    # ---- weights (resident), loaded after batch-0's x ----
    def load_w(name, d1, d2, dt):
        t_ = sb.tile([128, d1, d2], dt, tag=f"w_{name}")
        nc.gpsimd.dma_start(t_[:], io[name].ap())
        return t_

    def stage_weights_tm():
        global wka, wkb, wva, wvb, wra, wrb, woT8
        wka = load_w("wka", CC, C, FP8)
        wkb = load_w("wkb", CC, C, FP8)
        wva = load_w("wva", CC, C, FP8)
        wvb = load_w("wvb", CC, C, FP8)
        wra = load_w("wra", CC, C, FP8)
        wrb = load_w("wrb", CC, C, FP8)
        woT8 = load_w("woT8", CC, C, FP8)

    def stage_weights_cm():
        global cwk8, cwv8, cwr8
        cwk8 = load_w("cwk8", CC, H, FP8)
        cwv8 = load_w("cwv8", HC, C, FP8)
        cwr8 = load_w("cwr8", CC, C, FP8)

    def dr_fold(out_ps, wa, wb, xnB8, hh, th):
        """k/v/r: 4 DoubleRow matmuls, contraction 1024 = (aligned 512 +
        shifted 512), accumulating into out_ps [128, 512]."""
        t0 = 32 + th * 512
        for j in range(2):
            nc.tensor.matmul(out_ps[:], wa[:, 2 * j:2 * j + 2,
                                           hh * 128:(hh + 1) * 128],
                             xnB8[:, 2 * j:2 * j + 2, t0:t0 + 512],
                             start=(j == 0), stop=False, perf_mode=DR)
        for j in range(2):
            nc.tensor.matmul(out_ps[:], wb[:, 2 * j:2 * j + 2,
                                           hh * 128:(hh + 1) * 128],
                             xnB8[:, 2 * j:2 * j + 2, t0 - 1:t0 + 511],
                             start=False, stop=(j == 1), perf_mode=DR)

    def stage_K(b):
        """k/v/r matmuls + WKV chains -> srw(b) [128, CC, T] fp8."""
        xnB8 = xnB8_t[b]
        srw = srwp.tile([128, CC, T], FP8, tag="srw", name=f"srw{b}")
        for hh in range(CC):
            k_ps = ps2.tile([128, 1024], F32, tag="ps2", name=f"kps{b}_{hh}")
            for th in range(2):
                dr_fold(k_ps[:, th * 512:(th + 1) * 512], wka, wkb,
                        xnB8, hh, th)
            e_t = wkvp.tile([128, T], BF16, tag="e")
            nc.scalar.activation(e_t[:], k_ps[:], AF.Exp, scale=ISC,
                                 bias=nln64_t[:])
            r_ps = ps2.tile([128, 1024], F32, tag="ps2", name=f"rps{b}_{hh}")
            for th in range(2):
                dr_fold(r_ps[:, th * 512:(th + 1) * 512], wra, wrb,
                        xnB8, hh, th)
            sig = wkvp.tile([128, T], BF16, tag="sig")
            nc.scalar.activation(sig[:], r_ps[:], AF.Sigmoid, scale=ISC,
                                 bias=rb_c[hh][:])
            v_ps = ps2.tile([128, 1024], F32, tag="ps2", name=f"vps{b}_{hh}")
            for th in range(2):
                dr_fold(v_ps[:, th * 512:(th + 1) * 512], wva, wvb,
                        xnB8, hh, th)
            vcp = wkvp.tile([128, T], BF16, tag="vcp")
            nc.scalar.activation(vcp[:], v_ps[:], AF.Copy)
            ev = wkvp.tile([128, T], BF16, tag="ev")
            nc.gpsimd.tensor_tensor(ev[:], e_t[:], vcp[:], op=OP.mult)
            Pb = wkvp.tile([128, T + 1], BF16, tag="Pb")
            Qb = wkvp.tile([128, T + 1], F32, tag="Qb")
            nc.vector.memset(Pb[:, 0:1], 0.0)
            nc.vector.memset(Qb[:, 0:1], 0.0)
            db = delta_c[hh][:].to_broadcast((128, T))
            nc.vector.tensor_tensor_scan(Pb[:, 1:T + 1], db, ev[:],
                                         0.0, op0=OP.mult, op1=OP.add)
            nc.vector.tensor_tensor_scan(Qb[:, 1:T + 1], db, e_t[:],
                                         0.0, op0=OP.mult, op1=OP.add)
            # N' into ev, D' into Qb (slot t holds Q'_{t-1}), both in place
            nc.vector.scalar_tensor_tensor(ev[:], ev[:], eu_c[hh][:],
                                           Pb[:, 0:T], op0=OP.mult, op1=OP.add)
            nc.vector.scalar_tensor_tensor(Qb[:, 0:T], e_t[:], eu_c[hh][:],
                                           Qb[:, 0:T], op0=OP.mult, op1=OP.add)
            df = wkvp.tile([128, T], F32, tag="df")
            nc.vector.reciprocal_approx_fast(df[:], Qb[:, 0:T])
            nc.vector.scalar_tensor_tensor(ev[:], ev[:], 1.0, df[:],
                                           op0=OP.bypass, op1=OP.mult)
            nc.gpsimd.tensor_tensor(srw[:, hh, :], ev[:], sig[:], op=OP.mult)
        srw_t[b] = srw

    def stage_W(b):
        """Wo (fp8 DR, srw-stationary) + residual add in place: xa -> x1."""
        xa = xa_t[b]
        srw = srw_t[b]
        for n in range(NT):
            p_ = ps.tile([128, 512], F32, tag="ps", name=f"wops{b}_{n}")
            for j in range(2):
                nc.tensor.matmul(p_[:],
                                 srw[:, 2 * j:2 * j + 2,
                                     n * 128:(n + 1) * 128],
                                 woT8[:, 2 * j:2 * j + 2, :],
                                 start=(j == 0), stop=(j == 1), perf_mode=DR)
            nc.vector.scalar_tensor_tensor(xa[:, n, :], p_[:], 1.0 / 512.0,
                                           xa[:, n, :],
                                           op0=OP.mult, op1=OP.add)

    xn2B_t = {}

    def stage_L(b):
        """LN2 + xk2 mix (fp8); xr2 deferred to stage_L2."""
        xn2B = layer_norm(b, xa_t[b], 2)
        xn2B_t[b] = xn2B
        d2 = b2p.tile([128, CC, T], BF16, tag="d2", name=f"d2_{b}")
        xk2 = b2p.tile([128, CC, T], FP8, tag="xk2", name=f"xk2_{b}")
        for cc in range(CC):
            nc.gpsimd.tensor_tensor(d2[:, cc, :], xn2B[:, cc, 32:T + 32],
                                    xn2B[:, cc, 31:T + 31], op=OP.subtract)
        for cc in range(CC):
            nc.vector.scalar_tensor_tensor(xk2[:, cc, :], d2[:, cc, :],
                                           cmk_c[cc][:], xn2B[:, cc, 32:T + 32],
                                           op0=OP.mult, op1=OP.add)
        xk2_t[b] = xk2
        return d2

    def stage_L2(b, d2):
        xn2B = xn2B_t[b]
        xr2 = b2p.tile([128, CC, T], FP8, tag="xr2", name=f"xr2_{b}")
        for cc in range(CC):
            nc.vector.scalar_tensor_tensor(xr2[:, cc, :], d2[:, cc, :],
                                           cmr_c[cc][:], xn2B[:, cc, 32:T + 32],
                                           op0=OP.mult, op1=OP.add)
        xr2_t[b] = xr2

    def stage_M(b):
        """Channel mix: kk = relu(cWk xk2)^2 (fp8), rkv, residual, store."""
        xk2, xr2 = xk2_t[b], xr2_t[b]
        x1 = xa_t[b]
        yb = y_d[b].rearrange("(n p) c -> p n c", p=128)
        kk2 = cmp_.tile([128, HC, T], FP8, tag="kk2", name=f"kk2_{b}")
        rl = cmp_.tile([128, 2, T], BF16, tag="rl")
        for g in range(HC // 2):
            pps = []
            for u in range(2):
                hh = 2 * g + u
                p_ = ps2.tile([128, 1024], F32, tag="ps2",
                              name=f"kkps{b}_{hh}")
                for th in range(2):
                    for j in range(2):
                        nc.tensor.matmul(
                            p_[:, th * 512:(th + 1) * 512],
                            cwk8[:, 2 * j:2 * j + 2, hh * 128:(hh + 1) * 128],
                            xk2[:, 2 * j:2 * j + 2, th * 512:(th + 1) * 512],
                            start=(j == 0), stop=(j == 1), perf_mode=DR)
                pps.append((hh, p_))
            for u, (hh, p_) in enumerate(pps):
                nc.scalar.activation(rl[:, u, :], p_[:], AF.Relu, scale=ISC,
                                     bias=kkb_c[hh][:])
            for u, (hh, p_) in enumerate(pps):
                nc.scalar.activation(kk2[:, hh, :], rl[:, u, :], AF.Square)
        for n in range(NT):
            rp = ps.tile([128, 512], F32, tag="ps", name=f"rp{b}_{n}")
            for j in range(2):
                nc.tensor.matmul(rp[:],
                                 xr2[:, 2 * j:2 * j + 2,
                                     n * 128:(n + 1) * 128],
                                 cwr8[:, 2 * j:2 * j + 2, :],
                                 start=(j == 0), stop=(j == 1), perf_mode=DR)
            sig2 = outp.tile([128, 512], BF16, tag="sig2")
            nc.scalar.activation(sig2[:], rp[:], AF.Sigmoid, scale=ISC)
            kvp = ps.tile([128, 512], F32, tag="ps", name=f"kvp{b}_{n}")
            for j in range(HC // 2):
                nc.tensor.matmul(
                    kvp[:], kk2[:, 2 * j:2 * j + 2, n * 128:(n + 1) * 128],
                    cwv8[:, 2 * j:2 * j + 2, :],
                    start=(j == 0), stop=(j == HC // 2 - 1), perf_mode=DR)
            kvs = outp.tile([128, 512], F32, tag="kvs")
            nc.vector.tensor_tensor(kvs[:], kvp[:], sig2[:], op=OP.mult)
            nc.vector.scalar_tensor_tensor(kvs[:], kvs[:], ISC, x1[:, n, :],
                                           op0=OP.mult, op1=OP.add)
            nc.gpsimd.dma_start(yb[:, n, :], kvs[:])

    # ---- software pipeline over batches ----
    # PE queue order: K(0) | W(0) K(1) M(0) | W(1) K(2) M(1) | ... | M(bl-1)
    stage_A(0)
    stage_weights_tm()
    stage_weights_cm()
    stage_K(0)
    if bl > 1:
        stage_A(1)
    for b in range(bl):
        stage_W(b)
        d2 = stage_L(b)
        stage_L2(b, d2)
        if b + 1 < bl:
            stage_K(b + 1)
        stage_M(b)
        if b + 2 < bl:
            stage_A(b + 2)


def build_program(bl=BL):
    nc = bacc.Bacc("TRN2", target_bir_lowering=False, debug=False,
                   num_devices=NCORES)
    io = {}
    io["x"] = nc.dram_tensor("x", [bl, T, C], F32, kind="ExternalInput")
    io["y"] = nc.dram_tensor("y", [bl, T, C], F32, kind="ExternalOutput")
    for nm, d1, d2 in [("wka", CC, C), ("wkb", CC, C), ("wva", CC, C),
                       ("wvb", CC, C), ("wra", CC, C), ("wrb", CC, C),
                       ("woT8", CC, C), ("cwk8", CC, H), ("cwv8", HC, C),
                       ("cwr8", CC, C)]:
        io[nm] = nc.dram_tensor(nm, [128, d1, d2], FP8, kind="ExternalInput")
    for nm, n in [("delta", C), ("eu", C), ("rb", C),
                  ("cmk", C), ("cmr", C), ("kkb", H)]:
        io[nm] = nc.dram_tensor(nm, [n], F32, kind="ExternalInput")
    io["identT"] = nc.dram_tensor("identT", [128, 128], BF16,
                                  kind="ExternalInput")

    with tile.TileContext(nc) as tc:
        with ExitStack() as ctx:
            _emit(nc, tc, ctx, io, bl)
    nc.compile()
    return nc


def _pack8(w, scale=SC):  # [C_in, M] f32 -> [128, C_in//128, M] fp8e4m3
    ci, m = w.shape
    w8 = np.clip(w * scale, -240.0, 240.0).astype(ml_dtypes.float8_e4m3)
    return np.ascontiguousarray(w8.reshape(ci // 128, 128, m).transpose(1, 0, 2))


def host_params(inputs):
    """Host-side parameter prep (O(C^2) only)."""
    f32 = np.float32
    g1 = np.asarray(inputs["ln1_g"], f32)
    b1 = np.asarray(inputs["ln1_b"], f32)
    g2 = np.asarray(inputs["ln2_g"], f32)
    b2 = np.asarray(inputs["ln2_b"], f32)
    Wk = np.asarray(inputs["Wk"], f32)
    Wv = np.asarray(inputs["Wv"], f32)
    Wr = np.asarray(inputs["Wr"], f32)
    Wo = np.asarray(inputs["Wo"], f32)
    cWk = np.asarray(inputs["cWk"], f32)
    cWr = np.asarray(inputs["cWr"], f32)
    cWv = np.asarray(inputs["cWv"], f32)
    tmk = np.asarray(inputs["tm_k"], f32)
    tmv = np.asarray(inputs["tm_v"], f32)
    tmr = np.asarray(inputs["tm_r"], f32)

    # biases from LN betas land inside ACT bias terms; only the zero case is
    # supported (true for this model's init)
    assert np.allclose(Wk @ b1, 0.0, atol=1e-30), "nonzero ln1_b unsupported"
    assert np.allclose(Wv @ b1, 0.0, atol=1e-30), "nonzero ln1_b unsupported"
    assert np.allclose(cWr @ b2, 0.0, atol=1e-30), "nonzero ln2_b unsupported"

    p = {
        "wka": _pack8(Wk.T * (g1 * tmk)[:, None]),
        "wkb": _pack8(Wk.T * (g1 * (1.0 - tmk))[:, None]),
        "wva": _pack8(Wv.T * (g1 * tmv)[:, None], SCV),
        "wvb": _pack8(Wv.T * (g1 * (1.0 - tmv))[:, None], SCV),
        "wra": _pack8(Wr.T * (g1 * tmr)[:, None]),
        "wrb": _pack8(Wr.T * (g1 * (1.0 - tmr))[:, None]),
        "woT8": _pack8(Wo.T),
        "cwk8": _pack8(cWk.T * g2[:, None]),
        "cwv8": _pack8(cWv.T),
        "cwr8": _pack8(cWr.T * g2[:, None]),
        "delta": np.exp(-np.exp(np.asarray(inputs["time_decay"], f32))),
        "eu": np.exp(np.asarray(inputs["time_first"], f32)),
        "cmk": np.asarray(inputs["cm_k"], f32),
        "cmr": np.asarray(inputs["cm_r"], f32),
        "rb": (Wr @ b1).astype(f32),
        "kkb": (cWk @ b2).astype(f32),
        "identT": np.eye(128, dtype=f32).astype(ml_dtypes.bfloat16),
    }
    return p


_CACHE = {}


def kernel(**inputs):
    from concourse.bass_utils import run_bass_kernel_spmd

    if "nc" not in _CACHE:
        _CACHE["nc"] = build_program(BL)
    nc = _CACHE["nc"]

    p = host_params(inputs)
    x = np.asarray(inputs["x"], np.float32)
    in_maps = []
    for c in range(NCORES):
        m = dict(p)
        m["x"] = np.ascontiguousarray(x[c * BL:(c + 1) * BL])
        in_maps.append(m)
    res = run_bass_kernel_spmd(nc, in_maps, list(range(NCORES)))
    out = np.concatenate([res.results[c]["y"] for c in range(NCORES)], axis=0)
    return out.astype(np.float32)


# revision 46
# speedup vs baseline: 1.8704x; 1.0551x over previous
"""RWKV-v4 block (time-mix WKV attention + channel-mix GLU) on 8 Trainium2
NeuronCores, data-parallel over batch B.  v3: all matmuls fp8e4m3 DoubleRow
(2 contraction chunks per instruction, ~2x bf16 column rate), bf16 WKV chain
in STT form on DVE, software-pipelined across the 4 local batches.

Layouts per core (B_local=4, T=1024, C=512, H=2048):
  - layout A: [t(128p), n(8), c(512)] -- LayerNorm (bn_stats), residual adds,
    final store.
  - layout B: [c(128p), cc(4), t(1056)] -- WKV scan along free dim, matmul
    operands.  A->B via bf16 DMA transpose through a DRAM bounce (32 zero
    cols in front make the token shift an offset view), then fp8 quantize.

Weight scaling (fp8 weights packed x64 except Wv x8):
  k_raw = 64k   -> e  = exp(k_raw/64 - ln64) = e_true/64        (ACT bias)
  v_raw = 8v    -> ev = e*v_raw = e_true*v/8
  P' = P/8 (scan ev), Q' = Q/64 (scan e)
  N' = eu*ev + P'_{t-1} = N/8,  D' = eu*e + Q'_{t-1} = D/64
  y' = N'/D' = 8y  -> srw = y'*sig in fp8 (|8y*sig| < 40 ok)
  att_raw = (64*Wo)@srw = 512*att -> x1 = xa + att_raw/512      (STT)
  r_raw = 64r -> sigmoid(r_raw/64 + rb); cWk/cWr/cWv x64 likewise.
The time-mix token shift is folded into k/v/r weights (Wa = W*diag(g*tm),
Wb = W*diag(g*(1-tm))), the shifted moving operand is an offset view of the
same fp8 tile.  Channel-mix shifts stay explicit (output dim >> contraction).
"""

import numpy as np
import ml_dtypes
from contextlib import ExitStack

import concourse.bass as bass
import concourse.tile as tile
from concourse import bacc, mybir

B, T, C = 32, 1024, 512
H = 4 * C
NCORES = 8
BL = B // NCORES  # batches per core
NT = T // 128     # 8 t-subtiles per batch
CC = C // 128     # 4 channel chunks
HC = H // 128     # 16 hidden chunks
SC = 64.0         # fp8 weight scale
ISC = 1.0 / SC
SCV = 8.0         # Wv fp8 scale
LN64 = float(np.log(64.0))

F32 = mybir.dt.float32
BF16 = mybir.dt.bfloat16
FP8 = mybir.dt.float8e4
AX = mybir.AxisListType
OP = mybir.AluOpType
AF = mybir.ActivationFunctionType
DR = mybir.MatmulPerfMode.DoubleRow


def _emit(nc, tc, ctx, io, bl):
    x_d = io["x"].ap()
    y_d = io["y"].ap()

    def col(name, c0):  # [128,1] slice of a [N] dram vector
        return io[name].ap()[c0 * 128:(c0 + 1) * 128].rearrange(
            "(c one) -> c one", one=1)

    sb = ctx.enter_context(tc.tile_pool(name="sb", bufs=1))
    ps2 = ctx.enter_context(tc.tile_pool(name="ps2", bufs=3, space="PSUM"))
    ps = ctx.enter_context(tc.tile_pool(name="ps", bufs=2, space="PSUM"))
    dramp = ctx.enter_context(tc.tile_pool(name="dram", bufs=4, space="DRAM"))

    # ---- small consts first (cheap), then x(b0) so LN1 starts immediately;
    # weights stream in behind it in first-use order ----
    def vecload(name, n=CC, madd=None):
        ts_ = []
        for i in range(n):
            t_ = sb.tile([128, 1], F32, tag=f"v_{name}_{i}")
            nc.gpsimd.dma_start(t_[:], col(name, i))
            if madd is not None:
                nc.vector.tensor_scalar_add(t_[:], t_[:], madd)
            ts_.append(t_)
        return ts_

    eps_t = sb.tile([128, 1], F32, tag="eps")
    nc.vector.memset(eps_t[:], 1e-5)
    nln64_t = sb.tile([128, 1], F32, tag="nln64")
    nc.vector.memset(nln64_t[:], -LN64)
    identT = sb.tile([128, 128], BF16, tag="identT")
    nc.gpsimd.dma_start(identT[:], io["identT"].ap())
    zrow = sb.tile([32, C], BF16, tag="zrow")
    nc.vector.memset(zrow[:], 0.0)

    delta_c = vecload("delta")
    eu_c = vecload("eu")
    rb_c = vecload("rb")
    kkb_c = vecload("kkb", HC)
    cmk_c = vecload("cmk", madd=-1.0)
    cmr_c = vecload("cmr", madd=-1.0)

    # ---- per-batch pools ----
    xa_pool = ctx.enter_context(tc.tile_pool(name="xa", bufs=3))
    lnp = ctx.enter_context(tc.tile_pool(name="ln", bufs=1))
    bq = ctx.enter_context(tc.tile_pool(name="bq", bufs=1))
    bp = ctx.enter_context(tc.tile_pool(name="bp", bufs=2))
    b2p = ctx.enter_context(tc.tile_pool(name="b2p", bufs=1))
    wkvp = ctx.enter_context(tc.tile_pool(name="wkv", bufs=2))
    srwp = ctx.enter_context(tc.tile_pool(name="srw", bufs=1))
    cmp_ = ctx.enter_context(tc.tile_pool(name="cm", bufs=1))
    outp = ctx.enter_context(tc.tile_pool(name="out", bufs=2))

    xa_t = {}     # b -> [128, NT, 512] f32 (becomes x1 in place after Wo)
    xnB8_t = {}   # b -> [128, CC, 1056] fp8 (LN1, k/v/r folded matmuls)
    xk2_t = {}    # b -> [128, CC, T] fp8
    xr2_t = {}    # b -> [128, CC, T] fp8
    srw_t = {}    # b -> [128, CC, T] fp8

    def layer_norm(b, src_tile, which):
        """[128, NT, 512] f32 layout A -> xnB bf16 [128, CC, 1056] layout B
        (32 zero cols in front for the token shift)."""
        bnst = lnp.tile([128, NT, 6], F32, tag=f"bnst{which}")
        for n in range(NT):
            nc.vector.bn_stats(bnst[:, n, :], src_tile[:, n, :])
        mv = lnp.tile([128, NT, 2], F32, tag=f"mv{which}")
        for n in range(NT):
            nc.vector.bn_aggr(mv[:, n, :], bnst[:, n, :])
        sqv = lnp.tile([128, NT], F32, tag=f"sqv{which}")
        nc.scalar.activation(sqv[:], mv[:, :, 1], AF.Sqrt, bias=eps_t[:])
        rstd = lnp.tile([128, NT], F32, tag=f"rstd{which}")
        nc.vector.reciprocal(rstd[:], sqv[:])
        xn = lnp.tile([128, NT, 512], BF16, tag=f"xn{which}")
        for n in range(NT):
            nc.vector.tensor_scalar(xn[:, n, :], src_tile[:, n, :],
                                    mv[:, n, 0:1], rstd[:, n:n + 1],
                                    op0=OP.subtract, op1=OP.mult)
        # A->B on the PE: 32 [128,128] transposes through PSUM; the drain
        # copy quantizes to fp8 for LN1.  32 zero cols in front keep the
        # token shift an offset view.
        pool = bp if which == 1 else b2p
        dt = FP8 if which == 1 else BF16
        xnB = pool.tile([128, CC, T + 32], dt, tag=f"xnB{which}",
                        name=f"xnB{which}_{b}")
        for cc in range(CC):
            nc.vector.memset(xnB[:, cc, 0:32], 0.0)
        for n in range(NT):
            p_ = ps.tile([128, 512], F32, tag="ps", name=f"tp{which}{b}_{n}")
            pb = p_[:].bitcast(BF16)
            for cc in range(CC):
                nc.tensor.transpose(pb[:, cc * 128:(cc + 1) * 128],
                                    xn[:, n, cc * 128:(cc + 1) * 128],
                                    identT[:])
            nc.scalar.activation(
                xnB[:, :, 32 + n * 128:32 + (n + 1) * 128],
                pb[:, 0:512].rearrange("p (c t) -> p c t", c=CC),
                AF.Copy)
        return xnB

    def stage_A(b):
        """load x(b), LN1(b), fp8 quantize."""
        xb = x_d[b].rearrange("(n p) c -> p n c", p=128)
        xa = xa_pool.tile([128, NT, 512], F32, tag="xa", name=f"xa{b}")
        nc.sync.dma_start(xa[:], xb)
        xa_t[b] = xa
        xnB8_t[b] = layer_norm(b, xa, 1)

    # ---- weights (resident), loaded after batch-0's x ----
    def load_w(name, d1, d2, dt):
        t_ = sb.tile([128, d1, d2], dt, tag=f"w_{name}")
        nc.gpsimd.dma_start(t_[:], io[name].ap())
        return t_

    def stage_weights_tm():
        global wka, wkb, wva, wvb, wra, wrb, woT8
        wka = load_w("wka", CC, C, FP8)
        wkb = load_w("wkb", CC, C, FP8)
        wva = load_w("wva", CC, C, FP8)
        wvb = load_w("wvb", CC, C, FP8)
        wra = load_w("wra", CC, C, FP8)
        wrb = load_w("wrb", CC, C, FP8)
        woT8 = load_w("woT8", CC, C, FP8)

    def stage_weights_cm():
        global cwk8, cwv8, cwr8
        cwk8 = load_w("cwk8", CC, H, FP8)
        cwv8 = load_w("cwv8", HC, C, FP8)
        cwr8 = load_w("cwr8", CC, C, FP8)

    def dr_fold(out_ps, wa, wb, xnB8, hh, th):
        """k/v/r: 4 DoubleRow matmuls, contraction 1024 = (aligned 512 +
        shifted 512), accumulating into out_ps [128, 512]."""
        t0 = 32 + th * 512
        for j in range(2):
            nc.tensor.matmul(out_ps[:], wa[:, 2 * j:2 * j + 2,
                                           hh * 128:(hh + 1) * 128],
                             xnB8[:, 2 * j:2 * j + 2, t0:t0 + 512],
                             start=(j == 0), stop=False, perf_mode=DR)
        for j in range(2):
            nc.tensor.matmul(out_ps[:], wb[:, 2 * j:2 * j + 2,
                                           hh * 128:(hh + 1) * 128],
                             xnB8[:, 2 * j:2 * j + 2, t0 - 1:t0 + 511],
                             start=False, stop=(j == 1), perf_mode=DR)

    def stage_K(b):
        """k/v/r matmuls + WKV chains -> srw(b) [128, CC, T] fp8."""
        xnB8 = xnB8_t[b]
        srw = srwp.tile([128, CC, T], FP8, tag="srw", name=f"srw{b}")
        for hh in range(CC):
            k_ps = ps2.tile([128, 1024], F32, tag="ps2", name=f"kps{b}_{hh}")
            for th in range(2):
                dr_fold(k_ps[:, th * 512:(th + 1) * 512], wka, wkb,
                        xnB8, hh, th)
            e_t = wkvp.tile([128, T], BF16, tag="e")
            nc.scalar.activation(e_t[:], k_ps[:], AF.Exp, scale=ISC,
                                 bias=nln64_t[:])
            r_ps = ps2.tile([128, 1024], F32, tag="ps2", name=f"rps{b}_{hh}")
            for th in range(2):
                dr_fold(r_ps[:, th * 512:(th + 1) * 512], wra, wrb,
                        xnB8, hh, th)
            sig = wkvp.tile([128, T], BF16, tag="sig")
            nc.scalar.activation(sig[:], r_ps[:], AF.Sigmoid, scale=ISC,
                                 bias=rb_c[hh][:])
            v_ps = ps2.tile([128, 1024], F32, tag="ps2", name=f"vps{b}_{hh}")
            for th in range(2):
                dr_fold(v_ps[:, th * 512:(th + 1) * 512], wva, wvb,
                        xnB8, hh, th)
            vcp = wkvp.tile([128, T], BF16, tag="vcp")
            nc.scalar.activation(vcp[:], v_ps[:], AF.Copy)
            ev = wkvp.tile([128, T], BF16, tag="ev")
            nc.gpsimd.tensor_tensor(ev[:], e_t[:], vcp[:], op=OP.mult)
            Pb = wkvp.tile([128, T + 32], BF16, tag="Pb")
            Qb = wkvp.tile([128, T + 1], F32, tag="Qb")
            nc.vector.memset(Pb[:, 31:32], 0.0)
            nc.vector.memset(Qb[:, 0:1], 0.0)
            db = delta_c[hh][:].to_broadcast((128, T))
            nc.vector.tensor_tensor_scan(Pb[:, 32:T + 32], db, ev[:],
                                         0.0, op0=OP.mult, op1=OP.add)
            nc.vector.tensor_tensor_scan(Qb[:, 1:T + 1], db, e_t[:],
                                         0.0, op0=OP.mult, op1=OP.add)
            # N' into ev, D' into Qb (slot t holds Q'_{t-1}), both in place
            nc.vector.scalar_tensor_tensor(ev[:], ev[:], eu_c[hh][:],
                                           Pb[:, 31:T + 31],
                                           op0=OP.mult, op1=OP.add)
            nc.vector.scalar_tensor_tensor(Qb[:, 0:T], e_t[:], eu_c[hh][:],
                                           Qb[:, 0:T], op0=OP.mult, op1=OP.add)
            df = wkvp.tile([128, T], F32, tag="df")
            nc.vector.reciprocal_approx_fast(df[:], Qb[:, 0:T])
            nc.vector.scalar_tensor_tensor(ev[:], ev[:], 1.0, df[:],
                                           op0=OP.bypass, op1=OP.mult)
            nc.gpsimd.tensor_tensor(srw[:, hh, :], ev[:], sig[:], op=OP.mult)
        srw_t[b] = srw

    def stage_W(b):
        """Wo (fp8 DR, srw-stationary) + residual add in place: xa -> x1."""
        xa = xa_t[b]
        srw = srw_t[b]
        for n in range(NT):
            p_ = ps.tile([128, 512], F32, tag="ps", name=f"wops{b}_{n}")
            for j in range(2):
                nc.tensor.matmul(p_[:],
                                 srw[:, 2 * j:2 * j + 2,
                                     n * 128:(n + 1) * 128],
                                 woT8[:, 2 * j:2 * j + 2, :],
                                 start=(j == 0), stop=(j == 1), perf_mode=DR)
            nc.vector.scalar_tensor_tensor(xa[:, n, :], p_[:], 1.0 / 512.0,
                                           xa[:, n, :],
                                           op0=OP.mult, op1=OP.add)

    xn2B_t = {}

    def stage_L(b):
        """LN2 + xk2 mix (fp8); xr2 deferred to stage_L2."""
        xn2B = layer_norm(b, xa_t[b], 2)
        xn2B_t[b] = xn2B
        d2 = b2p.tile([128, CC, T], BF16, tag="d2", name=f"d2_{b}")
        xk2 = b2p.tile([128, CC, T], FP8, tag="xk2", name=f"xk2_{b}")
        for cc in range(CC):
            nc.gpsimd.tensor_tensor(d2[:, cc, :], xn2B[:, cc, 32:T + 32],
                                    xn2B[:, cc, 31:T + 31], op=OP.subtract)
        for cc in range(CC):
            nc.vector.scalar_tensor_tensor(xk2[:, cc, :], d2[:, cc, :],
                                           cmk_c[cc][:], xn2B[:, cc, 32:T + 32],
                                           op0=OP.mult, op1=OP.add)
        xk2_t[b] = xk2
        return d2

    def stage_L2(b, d2):
        xn2B = xn2B_t[b]
        xr2 = b2p.tile([128, CC, T], FP8, tag="xr2", name=f"xr2_{b}")
        for cc in range(CC):
            nc.vector.scalar_tensor_tensor(xr2[:, cc, :], d2[:, cc, :],
                                           cmr_c[cc][:], xn2B[:, cc, 32:T + 32],
                                           op0=OP.mult, op1=OP.add)
        xr2_t[b] = xr2

    def stage_M(b):
        """Channel mix: kk = relu(cWk xk2)^2 (fp8), rkv, residual, store."""
        xk2, xr2 = xk2_t[b], xr2_t[b]
        x1 = xa_t[b]
        yb = y_d[b].rearrange("(n p) c -> p n c", p=128)
        kk2 = cmp_.tile([128, HC, T], FP8, tag="kk2", name=f"kk2_{b}")
        rl = cmp_.tile([128, 2, T], BF16, tag="rl")
        for g in range(HC // 2):
            pps = []
            for u in range(2):
                hh = 2 * g + u
                p_ = ps2.tile([128, 1024], F32, tag="ps2",
                              name=f"kkps{b}_{hh}")
                for th in range(2):
                    for j in range(2):
                        nc.tensor.matmul(
                            p_[:, th * 512:(th + 1) * 512],
                            cwk8[:, 2 * j:2 * j + 2, hh * 128:(hh + 1) * 128],
                            xk2[:, 2 * j:2 * j + 2, th * 512:(th + 1) * 512],
                            start=(j == 0), stop=(j == 1), perf_mode=DR)
                pps.append((hh, p_))
            for u, (hh, p_) in enumerate(pps):
                nc.scalar.activation(rl[:, u, :], p_[:], AF.Relu, scale=ISC,
                                     bias=kkb_c[hh][:])
            for u, (hh, p_) in enumerate(pps):
                nc.scalar.activation(kk2[:, hh, :], rl[:, u, :], AF.Square)
        for n in range(NT):
            rp = ps.tile([128, 512], F32, tag="ps", name=f"rp{b}_{n}")
            for j in range(2):
                nc.tensor.matmul(rp[:],
                                 xr2[:, 2 * j:2 * j + 2,
                                     n * 128:(n + 1) * 128],
                                 cwr8[:, 2 * j:2 * j + 2, :],
                                 start=(j == 0), stop=(j == 1), perf_mode=DR)
            sig2 = outp.tile([128, 512], BF16, tag="sig2")
            nc.scalar.activation(sig2[:], rp[:], AF.Sigmoid, scale=ISC)
            kvp = ps.tile([128, 512], F32, tag="ps", name=f"kvp{b}_{n}")
            for j in range(HC // 2):
                nc.tensor.matmul(
                    kvp[:], kk2[:, 2 * j:2 * j + 2, n * 128:(n + 1) * 128],
                    cwv8[:, 2 * j:2 * j + 2, :],
                    start=(j == 0), stop=(j == HC // 2 - 1), perf_mode=DR)
            kvs = outp.tile([128, 512], F32, tag="kvs")
            nc.vector.tensor_tensor(kvs[:], kvp[:], sig2[:], op=OP.mult)
            nc.vector.scalar_tensor_tensor(kvs[:], kvs[:], ISC, x1[:, n, :],
                                           op0=OP.mult, op1=OP.add)
            nc.gpsimd.dma_start(yb[:, n, :], kvs[:])

    # ---- software pipeline over batches ----
    # PE queue order: K(0) | W(0) K(1) M(0) | W(1) K(2) M(1) | ... | M(bl-1)
    stage_A(0)
    stage_weights_tm()
    stage_weights_cm()
    stage_K(0)
    if bl > 1:
        stage_A(1)
    for b in range(bl):
        stage_W(b)
        d2 = stage_L(b)
        stage_L2(b, d2)
        if b + 1 < bl:
            stage_K(b + 1)
        stage_M(b)
        if b + 2 < bl:
            stage_A(b + 2)


def build_program(bl=BL):
    nc = bacc.Bacc("TRN2", target_bir_lowering=False, debug=False,
                   num_devices=NCORES)
    io = {}
    io["x"] = nc.dram_tensor("x", [bl, T, C], F32, kind="ExternalInput")
    io["y"] = nc.dram_tensor("y", [bl, T, C], F32, kind="ExternalOutput")
    for nm, d1, d2 in [("wka", CC, C), ("wkb", CC, C), ("wva", CC, C),
                       ("wvb", CC, C), ("wra", CC, C), ("wrb", CC, C),
                       ("woT8", CC, C), ("cwk8", CC, H), ("cwv8", HC, C),
                       ("cwr8", CC, C)]:
        io[nm] = nc.dram_tensor(nm, [128, d1, d2], FP8, kind="ExternalInput")
    for nm, n in [("delta", C), ("eu", C), ("rb", C),
                  ("cmk", C), ("cmr", C), ("kkb", H)]:
        io[nm] = nc.dram_tensor(nm, [n], F32, kind="ExternalInput")
    io["identT"] = nc.dram_tensor("identT", [128, 128], BF16,
                                  kind="ExternalInput")

    with tile.TileContext(nc) as tc:
        with ExitStack() as ctx:
            _emit(nc, tc, ctx, io, bl)
    nc.compile()
    return nc


def _pack8(w, scale=SC):  # [C_in, M] f32 -> [128, C_in//128, M] fp8e4m3
    ci, m = w.shape
    w8 = np.clip(w * scale, -240.0, 240.0).astype(ml_dtypes.float8_e4m3)
    return np.ascontiguousarray(w8.reshape(ci // 128, 128, m).transpose(1, 0, 2))


def host_params(inputs):
    """Host-side parameter prep (O(C^2) only)."""
    f32 = np.float32
    g1 = np.asarray(inputs["ln1_g"], f32)
    b1 = np.asarray(inputs["ln1_b"], f32)
    g2 = np.asarray(inputs["ln2_g"], f32)
    b2 = np.asarray(inputs["ln2_b"], f32)
    Wk = np.asarray(inputs["Wk"], f32)
    Wv = np.asarray(inputs["Wv"], f32)
    Wr = np.asarray(inputs["Wr"], f32)
    Wo = np.asarray(inputs["Wo"], f32)
    cWk = np.asarray(inputs["cWk"], f32)
    cWr = np.asarray(inputs["cWr"], f32)
    cWv = np.asarray(inputs["cWv"], f32)
    tmk = np.asarray(inputs["tm_k"], f32)
    tmv = np.asarray(inputs["tm_v"], f32)
    tmr = np.asarray(inputs["tm_r"], f32)

    # biases from LN betas land inside ACT bias terms; only the zero case is
    # supported (true for this model's init)
    assert np.allclose(Wk @ b1, 0.0, atol=1e-30), "nonzero ln1_b unsupported"
    assert np.allclose(Wv @ b1, 0.0, atol=1e-30), "nonzero ln1_b unsupported"
    assert np.allclose(cWr @ b2, 0.0, atol=1e-30), "nonzero ln2_b unsupported"

    p = {
        "wka": _pack8(Wk.T * (g1 * tmk)[:, None]),
        "wkb": _pack8(Wk.T * (g1 * (1.0 - tmk))[:, None]),
        "wva": _pack8(Wv.T * (g1 * tmv)[:, None], SCV),
        "wvb": _pack8(Wv.T * (g1 * (1.0 - tmv))[:, None], SCV),
        "wra": _pack8(Wr.T * (g1 * tmr)[:, None]),
        "wrb": _pack8(Wr.T * (g1 * (1.0 - tmr))[:, None]),
        "woT8": _pack8(Wo.T),
        "cwk8": _pack8(cWk.T * g2[:, None]),
        "cwv8": _pack8(cWv.T),
        "cwr8": _pack8(cWr.T * g2[:, None]),
        "delta": np.exp(-np.exp(np.asarray(inputs["time_decay"], f32))),
        "eu": np.exp(np.asarray(inputs["time_first"], f32)),
        "cmk": np.asarray(inputs["cm_k"], f32),
        "cmr": np.asarray(inputs["cm_r"], f32),
        "rb": (Wr @ b1).astype(f32),
        "kkb": (cWk @ b2).astype(f32),
        "identT": np.eye(128, dtype=f32).astype(ml_dtypes.bfloat16),
    }
    return p


_CACHE = {}


def kernel(**inputs):
    from concourse.bass_utils import run_bass_kernel_spmd

    if "nc" not in _CACHE:
        _CACHE["nc"] = build_program(BL)
    nc = _CACHE["nc"]

    p = host_params(inputs)
    x = np.asarray(inputs["x"], np.float32)
    in_maps = []
    for c in range(NCORES):
        m = dict(p)
        m["x"] = np.ascontiguousarray(x[c * BL:(c + 1) * BL])
        in_maps.append(m)
    res = run_bass_kernel_spmd(nc, in_maps, list(range(NCORES)))
    out = np.concatenate([res.results[c]["y"] for c in range(NCORES)], axis=0)
    return out.astype(np.float32)
